# revision 13
# baseline (speedup 1.0000x reference)
"""Multi-head attention (B=2, L=2048, C=1024, H=16, D=64) on 8 trn2 NeuronCores.

Sharding: core c -> (batch b = c//4, head-group g = c%4, 4 heads per group).
Tensor-parallel over heads: W_q/W_k/W_v column-sliced per group, W_o
row-sliced; each core returns a partial output projection for its batch and
the host sums the 4 group partials (the all-reduce of the hint, done on host
since the kernel contract is full-in/full-out).

Per-core device pipeline (all matmuls bf16 with fp32 PSUM accumulation):
  A) Kt = (k @ Wk)^T first (full-width accumulation; S needs all of K),
     then Qt per 512-query block with column-sliced xq DMAs so the
     attention phase starts as soon as kt + qt-block0 are ready (~26us)
     instead of after both full projections (~50us). V in natural
     [L, channels] layout with 4 interleaved ones-columns ([V_h | 1] per
     head) for the softmax row-sum trick.
  B) S^T tiles (Lk on partitions) per (lq-block, head); exp split between
     ScalarE (native Exp activation) and DVE (Schraudolph int16 bit-trick:
     bf16 bits of exp(x) ~ int16(x*128*log2e/8 + 16250.6), max rel err
     ~3.3% which washes out through the softmax normalization) so the two
     engines share the ~109us/core exp workload.
  C) Yext^T = [V_h | 1]^T @ P^T  (M=65: row 64 = softmax denominator),
     then row-normalize via DVE reciprocal (read straight from PSUM) +
     GPSIMD partition-broadcast.
  D) out_partial = Ybar @ Wo_g, fp32, DMA'd out.

b_k is dropped (adds a per-query constant to logits -> softmax invariant);
b_v is folded into the host-side constant (b_v @ W_o + b_o) since sum(P)=1.
"""

import os
import numpy as np
import ml_dtypes

import concourse.bass as bass
import concourse.mybir as mybir
from concourse import bacc
from concourse.tile import TileContext
from concourse.bass_utils import run_bass_kernel_spmd

BF16 = ml_dtypes.bfloat16
BF = mybir.dt.bfloat16
F32 = mybir.dt.float32
I16 = mybir.dt.int16

B, L, C = 2, 2048, 1024
H, D = 16, 64
NG = 4            # head-groups (cores per batch)
HPG = 4           # heads per group
GC = HPG * D      # 256 channels per group
KC = C // 128     # 8 contraction chunks for projections
LC = L // 128     # 16 L-chunks
NBLK = 4          # lq blocks of 512
BLKW = 512
VW = HPG * (D + 1)  # 260: interleaved [V_h | ones] columns

# Schraudolph exp-in-bf16-bits: bits16 = round(x*(128*log2e)*scale + B)
# with scale = 1/sqrt(D) = 1/8 folded in. B calibrated for round-to-nearest
# (hardware-verified semantics), max rel err 3.3%, mean +1.1%.
A_EXP = 128.0 / float(np.log(2.0)) / 8.0
B_EXP = 16250.6
# groups of each lq-block x head-pair handed to DVE instead of ScalarE
# (HW-measured: a [128,1536] group costs ~1.75us on DVE vs ~1.57us on
# ScalarE; 3.5 of 11 on average balances both engines near ~116us
# including their other work)
OFF_GROUPS_EVEN = (1, 4, 7, 9)
OFF_GROUPS_ODD = (2, 5, 8)

_CACHE = {}


def _build(debug_dump=False):
    nc = bacc.Bacc("TRN2", target_bir_lowering=False, debug=False, num_devices=8)
    dbg = {}
    if debug_dump:
        dbg["qt0"] = nc.declare_dram_parameter("d_qt0", [128, L], BF, isOutput=True)
        dbg["kt0"] = nc.declare_dram_parameter("d_kt0", [128, L], BF, isOutput=True)
        dbg["v0"] = nc.declare_dram_parameter("d_v0", [128, VW], BF, isOutput=True)
        dbg["pt"] = nc.declare_dram_parameter("d_pt", [128, 3 * BLKW], BF, isOutput=True)
        dbg["yext"] = nc.declare_dram_parameter("d_yext", [D + 1, BLKW], F32, isOutput=True)
        dbg["rbc"] = nc.declare_dram_parameter("d_rbc", [64, BLKW], F32, isOutput=True)

    xq = nc.declare_dram_parameter("xqT", [C, L], BF, isOutput=False)
    xk = nc.declare_dram_parameter("xkT", [C, L], BF, isOutput=False)
    xv = nc.declare_dram_parameter("xvT", [C, L], BF, isOutput=False)
    wq = nc.declare_dram_parameter("wq", [128, KC * GC], BF, isOutput=False)
    wk = nc.declare_dram_parameter("wk", [128, KC * GC], BF, isOutput=False)
    wv = nc.declare_dram_parameter("wv", [128, KC * VW], BF, isOutput=False)
    wo = nc.declare_dram_parameter("wo", [128, 2 * C], BF, isOutput=False)
    bq = nc.declare_dram_parameter("bq", [GC, 1], F32, isOutput=False)
    out = nc.declare_dram_parameter("out", [L, C], F32, isOutput=True)

    with TileContext(nc) as tc:
        with (
            tc.tile_pool(name="w", bufs=1) as wpool,
            tc.tile_pool(name="xb", bufs=1) as xbpool,
            tc.tile_pool(name="qx", bufs=2) as qxpool,
            tc.tile_pool(name="qk", bufs=1) as qkpool,
            tc.tile_pool(name="vsb", bufs=1) as vpool,
            tc.tile_pool(name="pt", bufs=24) as ptpool,
            tc.tile_pool(name="sm", bufs=3) as smpool,
            tc.tile_pool(name="ob", bufs=4) as opool,
            tc.tile_pool(name="ps", bufs=2, space="PSUM") as spsum,
            tc.tile_pool(name="py", bufs=2, space="PSUM") as ypsum,
        ):
            Ident = mybir.ActivationFunctionType.Identity
            ExpF = mybir.ActivationFunctionType.Exp
            scale = 1.0 / float(np.sqrt(D))

            # ---- weights / constants (gpsimd DMA queue; sync queue is
            # reserved for the big x streams in priority order) ----
            wk_sb = wpool.tile([128, KC * GC], BF, tag="wk")
            nc.gpsimd.dma_start(wk_sb[:], wk[:])
            wq_sb = wpool.tile([128, KC * GC], BF, tag="wq")
            nc.gpsimd.dma_start(wq_sb[:], wq[:])
            bq_sb = []
            for oc in range(2):
                t = wpool.tile([128, 1], F32, tag=f"bq{oc}")
                nc.gpsimd.dma_start(t[:], bq[oc * 128:(oc + 1) * 128, :])
                bq_sb.append(t)
            wv_sb = wpool.tile([128, KC * VW], BF, tag="wv")
            nc.gpsimd.dma_start(wv_sb[:], wv[:])
            wo_sb = wpool.tile([128, 2 * C], BF, tag="wo")
            nc.gpsimd.dma_start(wo_sb[:], wo[:])
            # ones-pattern: +1.0 at the interleaved ones-columns (65h+64)
            vpat = wpool.tile([128, VW], F32, tag="vpat")
            nc.vector.memset(vpat[:], 0.0)
            for h in range(HPG):
                nc.vector.memset(vpat[:, h * (D + 1) + D: h * (D + 1) + D + 1], 1.0)

            # One DMA queue is descriptor-serial (~200GB/s measured); the
            # big x streams alternate between the sync and gpsimd queues to
            # reach the ~358GB/s aggregate.
            def dma2(i, dst, src):
                eng = nc.sync if i % 2 == 0 else nc.gpsimd
                eng.dma_start(dst, src)

            # ---- K projection ----
            # oc0 accumulates FULL-width while the xk chunks stream in (its
            # matmuls trail the chunk arrivals), so kt-oc0 is ready right
            # after the xk DMA finishes. S(p0) = heads 0,1 only needs
            # kt-oc0 + qt-blk0, so the attention phase starts ~23us.
            # oc1 (only needed by S(p1) ~8us later) re-reads the resident
            # chunks column-blocked through the 2-bank y-ring, leaving the
            # spsum banks free for Q-blk0 and the first S groups.
            kt_sb = [
                qkpool.tile([128, L], BF, tag=f"kt{oc}", name=f"kt{oc}")
                for oc in range(2)
            ]
            xk_sb = []
            for kc in range(KC):
                t = xbpool.tile([128, L], BF, tag=f"xb{kc}", name="xkt")
                dma2(kc, t[:], xk[kc * 128:(kc + 1) * 128, :])
                xk_sb.append(t)
            ps_a = spsum.tile([128, 3 * BLKW], F32, tag="s", name="kpa")
            ps_b = ypsum.tile([128, BLKW], F32, tag="y", name="kpb")
            for kc in range(KC):
                lhs = wk_sb[:, kc * GC: kc * GC + 128]
                for lqb in range(NBLK):
                    tgt = (
                        ps_a[:, lqb * BLKW:(lqb + 1) * BLKW]
                        if lqb < 3 else ps_b[:]
                    )
                    nc.tensor.matmul(
                        tgt,
                        lhs,
                        xk_sb[kc][:, lqb * BLKW:(lqb + 1) * BLKW],
                        start=(kc == 0),
                        stop=(kc == KC - 1),
                    )
            nc.vector.tensor_copy(kt_sb[0][:, 0:3 * BLKW], ps_a[:])
            nc.scalar.copy(kt_sb[0][:, 3 * BLKW:L], ps_b[:])

            # ---- Q projection, per 512-col lq-block (column-sliced DMA,
            # one 3-bank spsum tile per block: oc0 cols 0:512, oc1 512:1024)
            qt_sb = [
                qkpool.tile([128, L], BF, tag=f"qt{oc}", name=f"qt{oc}")
                for oc in range(2)
            ]
            qslices = {}

            def emit_q_dma(blk):
                tiles = []
                for kc in range(KC):
                    xt = qxpool.tile([128, BLKW], BF, tag=f"qx{kc % 4}")
                    dma2(kc, xt[:],
                         xq[kc * 128:(kc + 1) * 128, blk * BLKW:(blk + 1) * BLKW])
                    tiles.append(xt)
                qslices[blk] = tiles

            def emit_q_blk(blk):
                qps = spsum.tile([128, 3 * BLKW], F32, tag="s", name=f"qps{blk}")
                tiles = qslices.pop(blk)
                for kc in range(KC):
                    for oc in range(2):
                        nc.tensor.matmul(
                            qps[:, oc * BLKW:(oc + 1) * BLKW],
                            wq_sb[:, kc * GC + oc * 128: kc * GC + (oc + 1) * 128],
                            tiles[kc][:],
                            start=(kc == 0),
                            stop=(kc == KC - 1),
                        )
                dst0 = qt_sb[0][:, blk * BLKW:(blk + 1) * BLKW]
                dst1 = qt_sb[1][:, blk * BLKW:(blk + 1) * BLKW]
                nc.vector.tensor_scalar_add(dst0, qps[:, 0:BLKW], bq_sb[0][:])
                nc.scalar.activation(
                    dst1, qps[:, BLKW:2 * BLKW], Ident, bias=bq_sb[1][:]
                )

            emit_q_dma(0)
            emit_q_blk(0)

            # K-oc1 column-blocked on the y-ring (chunks already resident)
            for lkb in range(NBLK):
                kp = ypsum.tile([128, BLKW], F32, tag="y", name=f"kp1_{lkb}")
                for kc in range(KC):
                    nc.tensor.matmul(
                        kp[:],
                        wk_sb[:, kc * GC + 128: kc * GC + 256],
                        xk_sb[kc][:, lkb * BLKW:(lkb + 1) * BLKW],
                        start=(kc == 0),
                        stop=(kc == KC - 1),
                    )
                dst = kt_sb[1][:, lkb * BLKW:(lkb + 1) * BLKW]
                if lkb % 2 == 0:
                    nc.vector.tensor_copy(dst, kp[:])
                else:
                    nc.scalar.copy(dst, kp[:])

            if debug_dump:
                nc.sync.dma_start(dbg["kt0"][:], kt_sb[0][:])

            # remaining q block slices, then xv (xv DMAs reuse the xk
            # tiles' SBUF via the shared xb tags, so they gate on K-proj's
            # last read of each chunk — keep them behind the qx slices)
            for blk in range(1, NBLK):
                emit_q_dma(blk)
            xv_sb = []
            for kc in range(KC):
                t = xbpool.tile([128, L], BF, tag=f"xb{kc}", name="xvt")
                dma2(kc, t[:], xv[kc * 128:(kc + 1) * 128, :])
                xv_sb.append(t)

            # ---- V projection (emitted inside main loop as filler) ----
            v_sb = []

            def emit_V():
                for lc in range(LC):
                    vps = ypsum.tile([128, BLKW], F32, tag="y", name="vps")
                    for kc in range(KC):
                        nc.tensor.matmul(
                            vps[:, 0:VW],
                            xv_sb[kc][:, lc * 128:(lc + 1) * 128],
                            wv_sb[:, kc * VW:(kc + 1) * VW],
                            start=(kc == 0),
                            stop=(kc == KC - 1),
                        )
                    vt = vpool.tile([128, VW], BF, tag=f"v{lc}", name="vt")
                    nc.vector.tensor_add(vt[:], vps[:, 0:VW], vpat[:])
                    v_sb.append(vt)
                if debug_dump:
                    nc.sync.dma_start(dbg["v0"][:], v_sb[0][:])

            # ---- main loop: attention + output projection ----
            ybar = [
                qkpool.tile([128, L], BF, tag=f"yb{oc}", name=f"yb{oc}")
                for oc in range(2)
            ]
            ptmaps = {}

            def emit_B(p):
                blk, hp = divmod(p, 2)
                lq0 = blk * BLKW
                units = [(hh, lk) for lk in range(LC) for hh in range(2)]
                ptloc = {}
                for u0 in range(0, len(units), 3):
                    grp = units[u0:u0 + 3]
                    w = len(grp) * BLKW
                    ps = spsum.tile([128, 3 * BLKW], F32, tag="s", name="pss")
                    for i, (hh, lk) in enumerate(grp):
                        h = 2 * hp + hh
                        oc = h // 2
                        r0 = (h % 2) * 64
                        nc.tensor.matmul(
                            ps[:, i * BLKW:(i + 1) * BLKW],
                            kt_sb[oc][r0:r0 + 64, lk * 128:(lk + 1) * 128],
                            qt_sb[oc][r0:r0 + 64, lq0:lq0 + BLKW],
                            start=True,
                            stop=True,
                        )
                    pt = ptpool.tile([128, 3 * BLKW], BF, tag="pt")
                    off = OFF_GROUPS_EVEN if p % 2 == 0 else OFF_GROUPS_ODD
                    if (u0 // 3) in off:
                        # DVE path: exp via Schraudolph bits, int16 written
                        # straight into the bf16 pt tile
                        nc.vector.tensor_scalar(
                            pt[:, 0:w].bitcast(I16),
                            ps[:, 0:w],
                            A_EXP,
                            B_EXP,
                            mybir.AluOpType.mult,
                            mybir.AluOpType.add,
                        )
                    else:
                        nc.scalar.activation(pt[:, 0:w], ps[:, 0:w], ExpF, scale=scale)
                    for i, (hh, lk) in enumerate(grp):
                        ptloc[(hh, lk)] = (pt, i * BLKW)
                ptmaps[p] = ptloc

            def emit_C(p):
                blk, hp = divmod(p, 2)
                lq0 = blk * BLKW
                ptloc = ptmaps.pop(p)
                yexts = [
                    ypsum.tile([128, BLKW], F32, tag="y", name="yext")
                    for _ in range(2)
                ]
                for lk in range(LC):
                    for hh in range(2):
                        h = 2 * hp + hh
                        pt, col = ptloc[(hh, lk)]
                        nc.tensor.matmul(
                            yexts[hh][0:D + 1, :],
                            v_sb[lk][:, h * (D + 1):(h + 1) * (D + 1)],
                            pt[:, col:col + BLKW],
                            start=(lk == 0),
                            stop=(lk == LC - 1),
                        )
                for hh in range(2):
                    h = 2 * hp + hh
                    yext = yexts[hh]
                    if debug_dump and p == 0 and hh == 0:
                        ydump = smpool.tile([D + 1, BLKW], F32, tag="ydump")
                        nc.vector.tensor_copy(ydump[:], yext[0:D + 1, :])
                        nc.sync.dma_start(dbg["yext"][:], ydump[:])
                    # rs copy to SBUF: reciprocal_approx_fast is a raw-bit
                    # DVE trick and must not read PSUM directly (HW-verified
                    # failure: denominators come out garbage)
                    rs = smpool.tile([1, BLKW], F32, tag="rs")
                    nc.vector.tensor_copy(rs[:], yext[D:D + 1, :])
                    rr = smpool.tile([1, BLKW], F32, tag="rr")
                    nc.vector.reciprocal_approx_fast(rr[:], rs[:])
                    rbc = smpool.tile([64, BLKW], F32, tag="rbc")
                    nc.gpsimd.partition_broadcast(rbc[:], rr[:])
                    if debug_dump and p == 0 and hh == 0:
                        nc.sync.dma_start(dbg["rbc"][:], rbc[:])
                    oc = h // 2
                    r0 = (h % 2) * 64
                    nc.vector.tensor_mul(
                        ybar[oc][r0:r0 + 64, lq0:lq0 + BLKW],
                        yext[0:D, :],
                        rbc[:],
                    )

            def emit_D(blk):
                for lc in range(4 * blk, 4 * blk + 4):
                    osb = opool.tile([128, C], F32, tag="osb")
                    for cc in range(2):
                        po = ypsum.tile([128, BLKW], F32, tag="y")
                        for kc2 in range(2):
                            nc.tensor.matmul(
                                po[:],
                                ybar[kc2][:, lc * 128:(lc + 1) * 128],
                                wo_sb[:, kc2 * C + cc * BLKW: kc2 * C + (cc + 1) * BLKW],
                                start=(kc2 == 0),
                                stop=(kc2 == 1),
                            )
                        nc.vector.tensor_copy(
                            osb[:, cc * BLKW:(cc + 1) * BLKW], po[:]
                        )
                    eng = nc.sync if lc % 2 == 0 else nc.gpsimd
                    eng.dma_start(out[lc * 128:(lc + 1) * 128, :], osb[:])

            emit_B(0)
            emit_B(1)
            emit_V()   # V projection: PE filler under the first S/exp phases
            emit_C(0)
            for p in range(2, 8):
                if p % 2 == 0:
                    # qt block for this pair of S phases (data arrives via
                    # the column-sliced xq DMAs queued behind xv)
                    emit_q_blk(p // 2)
                emit_B(p)
                emit_C(p - 1)
                if p in (3, 5, 7):
                    # D has a full block of slack: deprioritize it so it only
                    # fills PE gaps instead of preempting S^T production.
                    with tc.high_priority(offset=-400):
                        emit_D((p - 3) // 2)
            emit_C(7)
            emit_D(3)

            if debug_dump:
                nc.sync.dma_start(dbg["qt0"][:], qt_sb[0][:])

    nc.compile()
    return nc


def _get_nc():
    if "nc" not in _CACHE:
        _CACHE["nc"] = _build()
    return _CACHE["nc"]


def _prep_in_maps(q, k, v, Wq, bq, Wk, bv_unused, Wv, Wo):
    del bv_unused
    xT = {}
    for b in range(B):
        xT[("q", b)] = np.ascontiguousarray(q[b].T).astype(BF16)
        xT[("k", b)] = np.ascontiguousarray(k[b].T).astype(BF16)
        xT[("v", b)] = np.ascontiguousarray(v[b].T).astype(BF16)
    in_maps = []
    for c in range(8):
        b, g = c // NG, c % NG
        wv_g = np.zeros((C, VW), dtype=BF16)
        for h in range(HPG):
            wv_g[:, h * (D + 1):h * (D + 1) + D] = Wv[
                :, g * GC + h * D: g * GC + (h + 1) * D
            ].astype(BF16)
        def tile_w(w):
            # [C_or_GC rows, n cols] -> [128, KC_chunks * n]: chunk kc holds
            # rows kc*128..kc*128+128 side by side
            r, n = w.shape
            return np.ascontiguousarray(
                w.reshape(r // 128, 128, n).transpose(1, 0, 2).reshape(128, -1)
            )
        in_maps.append({
            "xqT": xT[("q", b)],
            "xkT": xT[("k", b)],
            "xvT": xT[("v", b)],
            "wq": tile_w(Wq[:, g * GC:(g + 1) * GC].astype(BF16)),
            "wk": tile_w(Wk[:, g * GC:(g + 1) * GC].astype(BF16)),
            "wv": tile_w(wv_g),
            "wo": tile_w(Wo[g * GC:(g + 1) * GC, :].astype(BF16)),
            "bq": bq[g * GC:(g + 1) * GC].reshape(GC, 1).astype(np.float32),
        })
    return in_maps


def kernel(q, k, v, Wq, bq, Wk, bk, Wv, bv, Wo, bo):
    q = np.asarray(q, dtype=np.float32)
    k = np.asarray(k, dtype=np.float32)
    v = np.asarray(v, dtype=np.float32)
    Wq = np.asarray(Wq, dtype=np.float32)
    bq = np.asarray(bq, dtype=np.float32)
    Wk = np.asarray(Wk, dtype=np.float32)
    Wv = np.asarray(Wv, dtype=np.float32)
    bv = np.asarray(bv, dtype=np.float32)
    Wo = np.asarray(Wo, dtype=np.float32)
    bo = np.asarray(bo, dtype=np.float32)

    nc = _get_nc()
    in_maps = _prep_in_maps(q, k, v, Wq, bq, Wk, None, Wv, Wo)

    trace = bool(int(os.environ.get("BASS_KERNEL_PROFILE", "0")))
    kwargs = {}
    if trace:
        try:
            from ntff_hook import install as _install_hook
            _install_hook()
        except Exception:
            pass
        kwargs = {"trace": True}
        td = os.environ.get("BASS_KERNEL_TRACE_DIR")
        if td:
            kwargs["tmpdir"] = td
    res = run_bass_kernel_spmd(nc, in_maps, core_ids=list(range(8)), **kwargs)
    _CACHE["last_exec_time_ns"] = res.exec_time_ns

    # host gather: sum group partials per batch, add folded bias
    cvec = (bv.astype(np.float64) @ Wo.astype(np.float64) + bo).astype(np.float32)
    full = np.empty((B, L, C), dtype=np.float32)
    for b in range(B):
        acc = res.results[b * NG]["out"].astype(np.float32)
        for g in range(1, NG):
            acc = acc + res.results[b * NG + g]["out"]
        full[b] = acc + cvec[None, :]
    return full


# revision 17
# speedup vs baseline: 1.0461x; 1.0461x over previous
"""Multi-head attention (B=2, L=2048, C=1024, H=16, D=64) on 8 trn2 NeuronCores.

Sharding: core c -> (batch b = c//4, head-group g = c%4, 4 heads per group).
Tensor-parallel over heads: W_q/W_k/W_v column-sliced per group, W_o
row-sliced; each core returns a partial output projection for its batch and
the host sums the 4 group partials (the all-reduce of the hint, done on host
since the kernel contract is full-in/full-out).

Per-core device pipeline (all matmuls bf16 with fp32 PSUM accumulation):
  A) Kt = (k @ Wk)^T first (full-width accumulation; S needs all of K),
     then Qt per 512-query block with column-sliced xq DMAs so the
     attention phase starts as soon as kt + qt-block0 are ready (~26us)
     instead of after both full projections (~50us). V in natural
     [L, channels] layout with 4 interleaved ones-columns ([V_h | 1] per
     head) for the softmax row-sum trick.
  B) S^T tiles (Lk on partitions) per (lq-block, head); exp split between
     ScalarE (native Exp activation) and DVE (Schraudolph int16 bit-trick:
     bf16 bits of exp(x) ~ int16(x*128*log2e/8 + 16250.6), max rel err
     ~3.3% which washes out through the softmax normalization) so the two
     engines share the ~109us/core exp workload.
  C) Yext^T = [V_h | 1]^T @ P^T  (M=65: row 64 = softmax denominator),
     then row-normalize via DVE reciprocal (read straight from PSUM) +
     GPSIMD partition-broadcast.
  D) out_partial = Ybar @ Wo_g, fp32, DMA'd out.

b_k is dropped (adds a per-query constant to logits -> softmax invariant);
b_v is folded into the host-side constant (b_v @ W_o + b_o) since sum(P)=1.
"""

import os
import numpy as np
import ml_dtypes

import concourse.bass as bass
import concourse.mybir as mybir
from concourse import bacc
from concourse.tile import TileContext
from concourse.bass_utils import run_bass_kernel_spmd

BF16 = ml_dtypes.bfloat16
BF = mybir.dt.bfloat16
F32 = mybir.dt.float32
I16 = mybir.dt.int16

B, L, C = 2, 2048, 1024
H, D = 16, 64
NG = 4            # head-groups (cores per batch)
HPG = 4           # heads per group
GC = HPG * D      # 256 channels per group
KC = C // 128     # 8 contraction chunks for projections
LC = L // 128     # 16 L-chunks
NBLK = 4          # lq blocks of 512
BLKW = 512
VW = HPG * (D + 1)  # 260: interleaved [V_h | ones] columns

# Schraudolph exp-in-bf16-bits: bits16 = round(x*(128*log2e)*scale + B)
# with scale = 1/sqrt(D) = 1/8 folded in. B calibrated for round-to-nearest
# (hardware-verified semantics), max rel err 3.3%, mean +1.1%.
A_EXP = 128.0 / float(np.log(2.0)) / 8.0
B_EXP = 16250.6
# groups of each lq-block x head-pair handed to DVE instead of ScalarE
# (HW-measured: a [128,1536] group costs ~1.75us on DVE vs ~1.57us on
# ScalarE; 3.5 of 11 on average balances both engines near ~116us
# including their other work)
OFF_GROUPS_EVEN = (1, 4, 7, 9)
OFF_GROUPS_ODD = (2, 5, 8)

_CACHE = {}


def _build(debug_dump=False):
    nc = bacc.Bacc("TRN2", target_bir_lowering=False, debug=False, num_devices=8)
    dbg = {}
    if debug_dump:
        dbg["qt0"] = nc.declare_dram_parameter("d_qt0", [128, L], BF, isOutput=True)
        dbg["kt0"] = nc.declare_dram_parameter("d_kt0", [128, L], BF, isOutput=True)
        dbg["v0"] = nc.declare_dram_parameter("d_v0", [128, VW], BF, isOutput=True)
        dbg["pt"] = nc.declare_dram_parameter("d_pt", [128, 3 * BLKW], BF, isOutput=True)
        dbg["yext"] = nc.declare_dram_parameter("d_yext", [D + 1, BLKW], F32, isOutput=True)
        dbg["rbc"] = nc.declare_dram_parameter("d_rbc", [64, BLKW], F32, isOutput=True)

    xq = nc.declare_dram_parameter("xqT", [C, L], BF, isOutput=False)
    xk = nc.declare_dram_parameter("xkT", [C, L], BF, isOutput=False)
    xv = nc.declare_dram_parameter("xvT", [C, L], BF, isOutput=False)
    wq = nc.declare_dram_parameter("wq", [128, KC * GC], BF, isOutput=False)
    wk = nc.declare_dram_parameter("wk", [128, KC * GC], BF, isOutput=False)
    wv = nc.declare_dram_parameter("wv", [128, KC * VW], BF, isOutput=False)
    wo = nc.declare_dram_parameter("wo", [128, 2 * C], BF, isOutput=False)
    bq = nc.declare_dram_parameter("bq", [GC, 1], F32, isOutput=False)
    out = nc.declare_dram_parameter("out", [L, C], F32, isOutput=True)

    with TileContext(nc) as tc:
        with (
            tc.tile_pool(name="w", bufs=1) as wpool,
            tc.tile_pool(name="xb", bufs=1) as xbpool,
            tc.tile_pool(name="xvs", bufs=2) as xvspool,
            tc.tile_pool(name="qx", bufs=2) as qxpool,
            tc.tile_pool(name="qk", bufs=1) as qkpool,
            tc.tile_pool(name="vsb", bufs=1) as vpool,
            tc.tile_pool(name="pt", bufs=22) as ptpool,
            tc.tile_pool(name="sm", bufs=2) as smpool,
            tc.tile_pool(name="ob", bufs=4) as opool,
            tc.tile_pool(name="ps", bufs=2, space="PSUM") as spsum,
            tc.tile_pool(name="py", bufs=2, space="PSUM") as ypsum,
        ):
            Ident = mybir.ActivationFunctionType.Identity
            ExpF = mybir.ActivationFunctionType.Exp
            scale = 1.0 / float(np.sqrt(D))

            # ---- weights / constants (gpsimd DMA queue; sync queue is
            # reserved for the big x streams in priority order) ----
            wk_sb = wpool.tile([128, KC * GC], BF, tag="wk")
            nc.sync.dma_start(wk_sb[:], wk[:])
            wq_sb = wpool.tile([128, KC * GC], BF, tag="wq")
            nc.sync.dma_start(wq_sb[:], wq[:])
            bq_sb = []
            for oc in range(2):
                t = wpool.tile([128, 1], F32, tag=f"bq{oc}")
                nc.gpsimd.dma_start(t[:], bq[oc * 128:(oc + 1) * 128, :])
                bq_sb.append(t)
            wv_sb = wpool.tile([128, KC * VW], BF, tag="wv")
            nc.gpsimd.dma_start(wv_sb[:], wv[:])
            wo_sb = wpool.tile([128, 2 * C], BF, tag="wo")
            nc.gpsimd.dma_start(wo_sb[:], wo[:])
            # ones-pattern: +1.0 at the interleaved ones-columns (65h+64)
            vpat = wpool.tile([128, VW], F32, tag="vpat")
            nc.vector.memset(vpat[:], 0.0)
            for h in range(HPG):
                nc.vector.memset(vpat[:, h * (D + 1) + D: h * (D + 1) + D + 1], 1.0)

            # One DMA queue is descriptor-serial (~200GB/s measured); the
            # big x streams alternate between the sync and gpsimd queues to
            # reach the ~358GB/s aggregate.
            def dma2(i, dst, src):
                eng = nc.sync if i % 2 == 0 else nc.gpsimd
                eng.dma_start(dst, src)

            # ---- K projection ----
            # oc0 accumulates FULL-width while the xk chunks stream in (its
            # matmuls trail the chunk arrivals), so kt-oc0 is ready right
            # after the xk DMA finishes. S(p0) = heads 0,1 only needs
            # kt-oc0 + qt-blk0, so the attention phase starts ~23us.
            # oc1 (only needed by S(p1) ~8us later) re-reads the resident
            # chunks column-blocked through the 2-bank y-ring, leaving the
            # spsum banks free for Q-blk0 and the first S groups.
            kt_sb = [
                qkpool.tile([128, L], BF, tag=f"kt{oc}", name=f"kt{oc}")
                for oc in range(2)
            ]
            xk_sb = []
            for kc in range(KC):
                t = xbpool.tile([128, L], BF, tag=f"xb{kc}", name="xkt")
                dma2(kc, t[:], xk[kc * 128:(kc + 1) * 128, :])
                xk_sb.append(t)
            ps_a = spsum.tile([128, 3 * BLKW], F32, tag="s", name="kpa")
            ps_b = ypsum.tile([128, BLKW], F32, tag="y", name="kpb")
            for kc in range(KC):
                lhs = wk_sb[:, kc * GC: kc * GC + 128]
                for lqb in range(NBLK):
                    tgt = (
                        ps_a[:, lqb * BLKW:(lqb + 1) * BLKW]
                        if lqb < 3 else ps_b[:]
                    )
                    nc.tensor.matmul(
                        tgt,
                        lhs,
                        xk_sb[kc][:, lqb * BLKW:(lqb + 1) * BLKW],
                        start=(kc == 0),
                        stop=(kc == KC - 1),
                    )
            nc.vector.tensor_copy(kt_sb[0][:, 0:3 * BLKW], ps_a[:])
            nc.scalar.copy(kt_sb[0][:, 3 * BLKW:L], ps_b[:])

            # ---- Q projection, per 512-col lq-block (column-sliced DMA,
            # one 3-bank spsum tile per block: oc0 cols 0:512, oc1 512:1024)
            qt_sb = [
                qkpool.tile([128, L], BF, tag=f"qt{oc}", name=f"qt{oc}")
                for oc in range(2)
            ]
            qslices = {}

            def emit_q_dma(blk):
                tiles = []
                for kc in range(KC):
                    xt = qxpool.tile([128, BLKW], BF, tag=f"qx{kc % 4}")
                    dma2(kc, xt[:],
                         xq[kc * 128:(kc + 1) * 128, blk * BLKW:(blk + 1) * BLKW])
                    tiles.append(xt)
                qslices[blk] = tiles

            def emit_q_blk(blk):
                qps = spsum.tile([128, 3 * BLKW], F32, tag="s", name=f"qps{blk}")
                tiles = qslices.pop(blk)
                for kc in range(KC):
                    for oc in range(2):
                        nc.tensor.matmul(
                            qps[:, oc * BLKW:(oc + 1) * BLKW],
                            wq_sb[:, kc * GC + oc * 128: kc * GC + (oc + 1) * 128],
                            tiles[kc][:],
                            start=(kc == 0),
                            stop=(kc == KC - 1),
                        )
                dst0 = qt_sb[0][:, blk * BLKW:(blk + 1) * BLKW]
                dst1 = qt_sb[1][:, blk * BLKW:(blk + 1) * BLKW]
                nc.vector.tensor_scalar_add(dst0, qps[:, 0:BLKW], bq_sb[0][:])
                nc.scalar.activation(
                    dst1, qps[:, BLKW:2 * BLKW], Ident, bias=bq_sb[1][:]
                )

            emit_q_dma(0)
            emit_q_blk(0)

            if debug_dump:
                nc.sync.dma_start(dbg["kt0"][:], kt_sb[0][:])

            # ---- xv loads as 512-wide column slices into their own small
            # pool: V-proj unit lc only needs slice column-block lc//4 of
            # every channel chunk, so the V projection streams right behind
            # these DMAs instead of gating on full-chunk residency ----
            xvs = {}
            for vb in range(NBLK):
                for kc in range(KC):
                    t = xvspool.tile([128, BLKW], BF, tag=f"xvs{kc}", name="xvst")
                    dma2(kc, t[:],
                         xv[kc * 128:(kc + 1) * 128, vb * BLKW:(vb + 1) * BLKW])
                    xvs[(kc, vb)] = t
            # remaining q block slices queue behind xv
            for blk in range(1, NBLK):
                emit_q_dma(blk)

            # ---- V projection (emitted inside main loop as filler) ----
            v_sb = []

            def emit_V():
                for lc in range(LC):
                    vps = ypsum.tile([128, BLKW], F32, tag="y", name="vps")
                    for kc in range(KC):
                        nc.tensor.matmul(
                            vps[:, 0:VW],
                            xvs[(kc, lc // 4)][:, (lc % 4) * 128:(lc % 4 + 1) * 128],
                            wv_sb[:, kc * VW:(kc + 1) * VW],
                            start=(kc == 0),
                            stop=(kc == KC - 1),
                        )
                    vt = vpool.tile([128, VW], BF, tag=f"v{lc}", name="vt")
                    nc.vector.tensor_add(vt[:], vps[:, 0:VW], vpat[:])
                    v_sb.append(vt)
                if debug_dump:
                    nc.sync.dma_start(dbg["v0"][:], v_sb[0][:])

            def emit_K_oc1():
                # K-oc1 column-blocked on the y-ring; chunks already
                # resident. Emitted AFTER B(0) so the scheduler slots these
                # matmuls into the exp-paced gaps of the first S phase
                # instead of blocking it.
                for lkb in range(NBLK):
                    kp = ypsum.tile([128, BLKW], F32, tag="y", name=f"kp1_{lkb}")
                    for kc in range(KC):
                        nc.tensor.matmul(
                            kp[:],
                            wk_sb[:, kc * GC + 128: kc * GC + 256],
                            xk_sb[kc][:, lkb * BLKW:(lkb + 1) * BLKW],
                            start=(kc == 0),
                            stop=(kc == KC - 1),
                        )
                    dst = kt_sb[1][:, lkb * BLKW:(lkb + 1) * BLKW]
                    if lkb % 2 == 0:
                        nc.vector.tensor_copy(dst, kp[:])
                    else:
                        nc.scalar.copy(dst, kp[:])

            # ---- main loop: attention + output projection ----
            ybar = [
                qkpool.tile([128, L], BF, tag=f"yb{oc}", name=f"yb{oc}")
                for oc in range(2)
            ]
            ptmaps = {}

            def emit_B(p):
                blk, hp = divmod(p, 2)
                lq0 = blk * BLKW
                units = [(hh, lk) for lk in range(LC) for hh in range(2)]
                ptloc = {}
                for u0 in range(0, len(units), 3):
                    grp = units[u0:u0 + 3]
                    w = len(grp) * BLKW
                    ps = spsum.tile([128, 3 * BLKW], F32, tag="s", name="pss")
                    for i, (hh, lk) in enumerate(grp):
                        h = 2 * hp + hh
                        oc = h // 2
                        r0 = (h % 2) * 64
                        nc.tensor.matmul(
                            ps[:, i * BLKW:(i + 1) * BLKW],
                            kt_sb[oc][r0:r0 + 64, lk * 128:(lk + 1) * 128],
                            qt_sb[oc][r0:r0 + 64, lq0:lq0 + BLKW],
                            start=True,
                            stop=True,
                        )
                    pt = ptpool.tile([128, 3 * BLKW], BF, tag="pt")
                    off = OFF_GROUPS_EVEN if p % 2 == 0 else OFF_GROUPS_ODD
                    if (u0 // 3) in off:
                        # DVE path: exp via Schraudolph bits, int16 written
                        # straight into the bf16 pt tile
                        nc.vector.tensor_scalar(
                            pt[:, 0:w].bitcast(I16),
                            ps[:, 0:w],
                            A_EXP,
                            B_EXP,
                            mybir.AluOpType.mult,
                            mybir.AluOpType.add,
                        )
                    else:
                        nc.scalar.activation(pt[:, 0:w], ps[:, 0:w], ExpF, scale=scale)
                    for i, (hh, lk) in enumerate(grp):
                        ptloc[(hh, lk)] = (pt, i * BLKW)
                ptmaps[p] = ptloc

            def emit_C(p):
                blk, hp = divmod(p, 2)
                lq0 = blk * BLKW
                ptloc = ptmaps.pop(p)
                yexts = [
                    ypsum.tile([128, BLKW], F32, tag="y", name="yext")
                    for _ in range(2)
                ]
                for lk in range(LC):
                    for hh in range(2):
                        h = 2 * hp + hh
                        pt, col = ptloc[(hh, lk)]
                        nc.tensor.matmul(
                            yexts[hh][0:D + 1, :],
                            v_sb[lk][:, h * (D + 1):(h + 1) * (D + 1)],
                            pt[:, col:col + BLKW],
                            start=(lk == 0),
                            stop=(lk == LC - 1),
                        )
                for hh in range(2):
                    h = 2 * hp + hh
                    yext = yexts[hh]
                    if debug_dump and p == 0 and hh == 0:
                        ydump = smpool.tile([D + 1, BLKW], F32, tag="ydump")
                        nc.vector.tensor_copy(ydump[:], yext[0:D + 1, :])
                        nc.sync.dma_start(dbg["yext"][:], ydump[:])
                    # rs copy to SBUF: reciprocal_approx_fast is a raw-bit
                    # DVE trick and must not read PSUM directly (HW-verified
                    # failure: denominators come out garbage)
                    rs = smpool.tile([1, BLKW], F32, tag="rs")
                    nc.vector.tensor_copy(rs[:], yext[D:D + 1, :])
                    rr = smpool.tile([1, BLKW], F32, tag="rr")
                    nc.vector.reciprocal_approx_fast(rr[:], rs[:])
                    rbc = smpool.tile([64, BLKW], F32, tag="rbc")
                    nc.gpsimd.partition_broadcast(rbc[:], rr[:])
                    if debug_dump and p == 0 and hh == 0:
                        nc.sync.dma_start(dbg["rbc"][:], rbc[:])
                    oc = h // 2
                    r0 = (h % 2) * 64
                    nc.vector.tensor_mul(
                        ybar[oc][r0:r0 + 64, lq0:lq0 + BLKW],
                        yext[0:D, :],
                        rbc[:],
                    )

            def emit_D(blk):
                for lc in range(4 * blk, 4 * blk + 4):
                    osb = opool.tile([128, C], F32, tag="osb")
                    for cc in range(2):
                        po = ypsum.tile([128, BLKW], F32, tag="y")
                        for kc2 in range(2):
                            nc.tensor.matmul(
                                po[:],
                                ybar[kc2][:, lc * 128:(lc + 1) * 128],
                                wo_sb[:, kc2 * C + cc * BLKW: kc2 * C + (cc + 1) * BLKW],
                                start=(kc2 == 0),
                                stop=(kc2 == 1),
                            )
                        nc.vector.tensor_copy(
                            osb[:, cc * BLKW:(cc + 1) * BLKW], po[:]
                        )
                    eng = nc.sync if lc % 2 == 0 else nc.gpsimd
                    eng.dma_start(out[lc * 128:(lc + 1) * 128, :], osb[:])

            emit_B(0)
            emit_K_oc1()
            emit_B(1)
            emit_V()   # V projection: PE filler under the first S/exp phases
            emit_C(0)
            for p in range(2, 8):
                if p % 2 == 0:
                    # qt block for this pair of S phases (data arrives via
                    # the column-sliced xq DMAs queued behind xv)
                    emit_q_blk(p // 2)
                emit_B(p)
                emit_C(p - 1)
                if p in (3, 5, 7):
                    # D has a full block of slack: deprioritize it so it only
                    # fills PE gaps instead of preempting S^T production.
                    with tc.high_priority(offset=-400):
                        emit_D((p - 3) // 2)
            emit_C(7)
            emit_D(3)

            if debug_dump:
                nc.sync.dma_start(dbg["qt0"][:], qt_sb[0][:])

    nc.compile()
    return nc


def _get_nc():
    if "nc" not in _CACHE:
        _CACHE["nc"] = _build()
    return _CACHE["nc"]


def _prep_in_maps(q, k, v, Wq, bq, Wk, bv_unused, Wv, Wo):
    del bv_unused
    xT = {}
    for b in range(B):
        xT[("q", b)] = np.ascontiguousarray(q[b].T).astype(BF16)
        xT[("k", b)] = np.ascontiguousarray(k[b].T).astype(BF16)
        xT[("v", b)] = np.ascontiguousarray(v[b].T).astype(BF16)
    in_maps = []
    for c in range(8):
        b, g = c // NG, c % NG
        wv_g = np.zeros((C, VW), dtype=BF16)
        for h in range(HPG):
            wv_g[:, h * (D + 1):h * (D + 1) + D] = Wv[
                :, g * GC + h * D: g * GC + (h + 1) * D
            ].astype(BF16)
        def tile_w(w):
            # [C_or_GC rows, n cols] -> [128, KC_chunks * n]: chunk kc holds
            # rows kc*128..kc*128+128 side by side
            r, n = w.shape
            return np.ascontiguousarray(
                w.reshape(r // 128, 128, n).transpose(1, 0, 2).reshape(128, -1)
            )
        in_maps.append({
            "xqT": xT[("q", b)],
            "xkT": xT[("k", b)],
            "xvT": xT[("v", b)],
            "wq": tile_w(Wq[:, g * GC:(g + 1) * GC].astype(BF16)),
            "wk": tile_w(Wk[:, g * GC:(g + 1) * GC].astype(BF16)),
            "wv": tile_w(wv_g),
            "wo": tile_w(Wo[g * GC:(g + 1) * GC, :].astype(BF16)),
            "bq": bq[g * GC:(g + 1) * GC].reshape(GC, 1).astype(np.float32),
        })
    return in_maps


def kernel(q, k, v, Wq, bq, Wk, bk, Wv, bv, Wo, bo):
    q = np.asarray(q, dtype=np.float32)
    k = np.asarray(k, dtype=np.float32)
    v = np.asarray(v, dtype=np.float32)
    Wq = np.asarray(Wq, dtype=np.float32)
    bq = np.asarray(bq, dtype=np.float32)
    Wk = np.asarray(Wk, dtype=np.float32)
    Wv = np.asarray(Wv, dtype=np.float32)
    bv = np.asarray(bv, dtype=np.float32)
    Wo = np.asarray(Wo, dtype=np.float32)
    bo = np.asarray(bo, dtype=np.float32)

    nc = _get_nc()
    in_maps = _prep_in_maps(q, k, v, Wq, bq, Wk, None, Wv, Wo)

    trace = bool(int(os.environ.get("BASS_KERNEL_PROFILE", "0")))
    kwargs = {}
    if trace:
        try:
            from ntff_hook import install as _install_hook
            _install_hook()
        except Exception:
            pass
        kwargs = {"trace": True}
        td = os.environ.get("BASS_KERNEL_TRACE_DIR")
        if td:
            kwargs["tmpdir"] = td
    res = run_bass_kernel_spmd(nc, in_maps, core_ids=list(range(8)), **kwargs)
    _CACHE["last_exec_time_ns"] = res.exec_time_ns

    # host gather: sum group partials per batch, add folded bias
    cvec = (bv.astype(np.float64) @ Wo.astype(np.float64) + bo).astype(np.float32)
    full = np.empty((B, L, C), dtype=np.float32)
    for b in range(B):
        acc = res.results[b * NG]["out"].astype(np.float32)
        for g in range(1, NG):
            acc = acc + res.results[b * NG + g]["out"]
        full[b] = acc + cvec[None, :]
    return full


# revision 22
# speedup vs baseline: 1.1304x; 1.0805x over previous
"""Multi-head attention (B=2, L=2048, C=1024, H=16, D=64) on 8 trn2 NeuronCores.

Sharding: core c -> (batch b = c//4, head-group g = c%4, 4 heads per group).
Tensor-parallel over heads: W_q/W_k/W_v column-sliced per group, W_o
row-sliced; each core returns a partial output projection for its batch and
the host sums the 4 group partials (the all-reduce of the hint, done on host
since the kernel contract is full-in/full-out).

Per-core device pipeline (all matmuls bf16 with fp32 PSUM accumulation):
  A) Kt = (k @ Wk)^T first (full-width accumulation; S needs all of K),
     then Qt per 512-query block with column-sliced xq DMAs so the
     attention phase starts as soon as kt + qt-block0 are ready (~26us)
     instead of after both full projections (~50us). V in natural
     [L, channels] layout with 4 interleaved ones-columns ([V_h | 1] per
     head) for the softmax row-sum trick.
  B) S^T tiles (Lk on partitions) per (lq-block, head); exp split between
     ScalarE (native Exp activation) and DVE (Schraudolph int16 bit-trick:
     bf16 bits of exp(x) ~ int16(x*128*log2e/8 + 16250.6), max rel err
     ~3.3% which washes out through the softmax normalization) so the two
     engines share the ~109us/core exp workload.
  C) Yext^T = [V_h | 1]^T @ P^T  (M=65: row 64 = softmax denominator),
     then row-normalize via DVE reciprocal (read straight from PSUM) +
     GPSIMD partition-broadcast.
  D) out_partial = Ybar @ Wo_g, fp32, DMA'd out.

b_k is dropped (adds a per-query constant to logits -> softmax invariant);
b_v is folded into the host-side constant (b_v @ W_o + b_o) since sum(P)=1.
"""

import os
import numpy as np
import ml_dtypes

import concourse.bass as bass
import concourse.mybir as mybir
from concourse import bacc
from concourse.tile import TileContext
from concourse.bass_utils import run_bass_kernel_spmd

BF16 = ml_dtypes.bfloat16
BF = mybir.dt.bfloat16
F32 = mybir.dt.float32
I16 = mybir.dt.int16

B, L, C = 2, 2048, 1024
H, D = 16, 64
NG = 4            # head-groups (cores per batch)
HPG = 4           # heads per group
GC = HPG * D      # 256 channels per group
KC = C // 128     # 8 contraction chunks for projections
LC = L // 128     # 16 L-chunks
NBLK = 4          # lq blocks of 512
BLKW = 512
VW = HPG * (D + 1)  # 260: interleaved [V_h | ones] columns

# Schraudolph exp-in-bf16-bits: bits16 = round(x*(128*log2e)*scale + B)
# with scale = 1/sqrt(D) = 1/8 folded in. B calibrated for round-to-nearest
# (hardware-verified semantics), max rel err 3.3%, mean +1.1%.
A_EXP = 128.0 / float(np.log(2.0)) / 8.0
B_EXP = 16250.6
# groups of each lq-block x head-pair handed to DVE instead of ScalarE
# (HW-measured: a [128,1536] group costs ~1.75us on DVE vs ~1.57us on
# ScalarE; 3.5 of 11 on average balances both engines near ~116us
# including their other work)
OFF_GROUPS_EVEN = (1, 4, 7, 9)
OFF_GROUPS_ODD = (2, 5, 8)

_CACHE = {}


def _build(debug_dump=False):
    nc = bacc.Bacc("TRN2", target_bir_lowering=False, debug=False, num_devices=8)
    dbg = {}
    if debug_dump:
        dbg["qt0"] = nc.declare_dram_parameter("d_qt0", [128, L], BF, isOutput=True)
        dbg["kt0"] = nc.declare_dram_parameter("d_kt0", [128, L], BF, isOutput=True)
        dbg["v0"] = nc.declare_dram_parameter("d_v0", [128, VW], BF, isOutput=True)
        dbg["pt"] = nc.declare_dram_parameter("d_pt", [128, 3 * BLKW], BF, isOutput=True)
        dbg["yext"] = nc.declare_dram_parameter("d_yext", [D + 1, BLKW], F32, isOutput=True)
        dbg["rbc"] = nc.declare_dram_parameter("d_rbc", [64, BLKW], F32, isOutput=True)

    xq = nc.declare_dram_parameter("xqT", [C, L], BF, isOutput=False)
    xk = nc.declare_dram_parameter("xkT", [C, L], BF, isOutput=False)
    xv = nc.declare_dram_parameter("xvT", [C, L], BF, isOutput=False)
    wq = nc.declare_dram_parameter("wq", [128, KC * GC], BF, isOutput=False)
    wk = nc.declare_dram_parameter("wk", [128, KC * GC], BF, isOutput=False)
    wv = nc.declare_dram_parameter("wv", [128, KC * VW], BF, isOutput=False)
    wo = nc.declare_dram_parameter("wo", [128, 2 * C], BF, isOutput=False)
    bq = nc.declare_dram_parameter("bq", [GC, 1], F32, isOutput=False)
    out = nc.declare_dram_parameter("out", [L, C], F32, isOutput=True)

    with TileContext(nc) as tc:
        with (
            tc.tile_pool(name="w", bufs=1) as wpool,
            tc.tile_pool(name="xb", bufs=1) as xbpool,
            tc.tile_pool(name="xvs", bufs=2) as xvspool,
            tc.tile_pool(name="qx", bufs=2) as qxpool,
            tc.tile_pool(name="qk", bufs=1) as qkpool,
            tc.tile_pool(name="vsb", bufs=1) as vpool,
            tc.tile_pool(name="pt", bufs=22) as ptpool,
            tc.tile_pool(name="sm", bufs=2) as smpool,
            tc.tile_pool(name="ob", bufs=4) as opool,
            tc.tile_pool(name="ps", bufs=2, space="PSUM") as spsum,
            tc.tile_pool(name="py", bufs=2, space="PSUM") as ypsum,
        ):
            Ident = mybir.ActivationFunctionType.Identity
            ExpF = mybir.ActivationFunctionType.Exp
            scale = 1.0 / float(np.sqrt(D))

            # ---- weights / constants (gpsimd DMA queue; sync queue is
            # reserved for the big x streams in priority order) ----
            wk_sb = wpool.tile([128, KC * GC], BF, tag="wk")
            nc.sync.dma_start(wk_sb[:], wk[:])
            wq_sb = wpool.tile([128, KC * GC], BF, tag="wq")
            nc.sync.dma_start(wq_sb[:], wq[:])
            bq_sb = []
            for oc in range(2):
                t = wpool.tile([128, 1], F32, tag=f"bq{oc}")
                nc.gpsimd.dma_start(t[:], bq[oc * 128:(oc + 1) * 128, :])
                bq_sb.append(t)
            wv_sb = wpool.tile([128, KC * VW], BF, tag="wv")
            wo_sb = wpool.tile([128, 2 * C], BF, tag="wo")
            # ones-pattern: +1.0 at the interleaved ones-columns (65h+64)
            vpat = wpool.tile([128, VW], F32, tag="vpat")
            nc.vector.memset(vpat[:], 0.0)
            for h in range(HPG):
                nc.vector.memset(vpat[:, h * (D + 1) + D: h * (D + 1) + D + 1], 1.0)

            # The sync queue uses hardware DGE (~200GB/s); the gpsimd queue
            # is software-DGE and only manages ~60-100GB/s. Split the big x
            # streams ~3:1 so both queues finish together.
            def dma2(i, dst, src):
                eng = nc.gpsimd if i % 4 == 3 else nc.sync
                eng.dma_start(dst, src)

            # ---- K projection ----
            # oc0 accumulates FULL-width while the xk chunks stream in (its
            # matmuls trail the chunk arrivals), so kt-oc0 is ready right
            # after the xk DMA finishes. S(p0) = heads 0,1 only needs
            # kt-oc0 + qt-blk0, so the attention phase starts ~23us.
            # oc1 (only needed by S(p1) ~8us later) re-reads the resident
            # chunks column-blocked through the 2-bank y-ring, leaving the
            # spsum banks free for Q-blk0 and the first S groups.
            kt_sb = [
                qkpool.tile([128, L], BF, tag=f"kt{oc}", name=f"kt{oc}")
                for oc in range(2)
            ]
            xk_sb = []
            for kc in range(KC):
                t = xbpool.tile([128, L], BF, tag=f"xb{kc}", name="xkt")
                dma2(kc, t[:], xk[kc * 128:(kc + 1) * 128, :])
                xk_sb.append(t)
            ps_a = spsum.tile([128, 3 * BLKW], F32, tag="s", name="kpa")
            ps_b = ypsum.tile([128, BLKW], F32, tag="y", name="kpb")
            for kc in range(KC):
                lhs = wk_sb[:, kc * GC: kc * GC + 128]
                for lqb in range(NBLK):
                    tgt = (
                        ps_a[:, lqb * BLKW:(lqb + 1) * BLKW]
                        if lqb < 3 else ps_b[:]
                    )
                    nc.tensor.matmul(
                        tgt,
                        lhs,
                        xk_sb[kc][:, lqb * BLKW:(lqb + 1) * BLKW],
                        start=(kc == 0),
                        stop=(kc == KC - 1),
                    )
            nc.vector.tensor_copy(kt_sb[0][:, 0:3 * BLKW], ps_a[:])
            nc.scalar.copy(kt_sb[0][:, 3 * BLKW:L], ps_b[:])

            # ---- Q projection, per 512-col lq-block (column-sliced DMA,
            # one 3-bank spsum tile per block: oc0 cols 0:512, oc1 512:1024)
            qt_sb = [
                qkpool.tile([128, L], BF, tag=f"qt{oc}", name=f"qt{oc}")
                for oc in range(2)
            ]
            qslices = {}

            def emit_q_dma(blk):
                tiles = []
                for kc in range(KC):
                    xt = qxpool.tile([128, BLKW], BF, tag=f"qx{kc % 4}")
                    dma2(kc, xt[:],
                         xq[kc * 128:(kc + 1) * 128, blk * BLKW:(blk + 1) * BLKW])
                    tiles.append(xt)
                qslices[blk] = tiles

            def emit_q_blk(blk):
                qps = spsum.tile([128, 3 * BLKW], F32, tag="s", name=f"qps{blk}")
                tiles = qslices.pop(blk)
                for kc in range(KC):
                    for oc in range(2):
                        nc.tensor.matmul(
                            qps[:, oc * BLKW:(oc + 1) * BLKW],
                            wq_sb[:, kc * GC + oc * 128: kc * GC + (oc + 1) * 128],
                            tiles[kc][:],
                            start=(kc == 0),
                            stop=(kc == KC - 1),
                        )
                dst0 = qt_sb[0][:, blk * BLKW:(blk + 1) * BLKW]
                dst1 = qt_sb[1][:, blk * BLKW:(blk + 1) * BLKW]
                nc.vector.tensor_scalar_add(dst0, qps[:, 0:BLKW], bq_sb[0][:])
                nc.scalar.activation(
                    dst1, qps[:, BLKW:2 * BLKW], Ident, bias=bq_sb[1][:]
                )

            emit_q_dma(0)
            emit_q_blk(0)

            if debug_dump:
                nc.sync.dma_start(dbg["kt0"][:], kt_sb[0][:])

            # V/O weights on the fast queue once the startup-critical
            # streams are behind them
            nc.sync.dma_start(wv_sb[:], wv[:])
            nc.sync.dma_start(wo_sb[:], wo[:])

            # ---- xv loads as 512-wide column slices into their own small
            # pool: V-proj unit lc only needs slice column-block lc//4 of
            # every channel chunk, so the V projection streams right behind
            # these DMAs instead of gating on full-chunk residency ----
            xvs = {}
            for vb in range(NBLK):
                for kc in range(KC):
                    t = xvspool.tile([128, BLKW], BF, tag=f"xvs{kc}", name="xvst")
                    dma2(kc, t[:],
                         xv[kc * 128:(kc + 1) * 128, vb * BLKW:(vb + 1) * BLKW])
                    xvs[(kc, vb)] = t
            # q block-1 slices behind xv on the qx tags; blocks 2-3 reuse
            # the xvs tags (free once the V projection consumes them)
            emit_q_dma(1)
            for blk in (2, 3):
                tiles = []
                for kc in range(KC):
                    t = xvspool.tile([128, BLKW], BF, tag=f"xvs{kc}", name="qxl")
                    dma2(kc, t[:],
                         xq[kc * 128:(kc + 1) * 128, blk * BLKW:(blk + 1) * BLKW])
                    tiles.append(t)
                qslices[blk] = tiles

            # dummy partition_broadcast: forces the GPSIMD custom-op
            # library load (~8us) to happen here, overlapped with the DMA
            # phase, instead of stalling the first softmax normalize
            dmb_i = wpool.tile([1, 8], F32, tag="dmb_i")
            nc.vector.memset(dmb_i[:], 0.0)
            dmb_o = wpool.tile([2, 8], F32, tag="dmb_o")
            nc.gpsimd.partition_broadcast(dmb_o[:], dmb_i[:])

            # ---- V projection (emitted inside main loop as filler) ----
            v_sb = []

            def emit_V():
                for lc in range(LC):
                    vps = ypsum.tile([128, BLKW], F32, tag="y", name="vps")
                    for kc in range(KC):
                        nc.tensor.matmul(
                            vps[:, 0:VW],
                            xvs[(kc, lc // 4)][:, (lc % 4) * 128:(lc % 4 + 1) * 128],
                            wv_sb[:, kc * VW:(kc + 1) * VW],
                            start=(kc == 0),
                            stop=(kc == KC - 1),
                        )
                    vt = vpool.tile([128, VW], BF, tag=f"v{lc}", name="vt")
                    nc.vector.tensor_add(vt[:], vps[:, 0:VW], vpat[:])
                    v_sb.append(vt)
                if debug_dump:
                    nc.sync.dma_start(dbg["v0"][:], v_sb[0][:])

            def emit_K_oc1():
                # K-oc1 column-blocked on the y-ring; chunks already
                # resident. Emitted AFTER B(0) so the scheduler slots these
                # matmuls into the exp-paced gaps of the first S phase
                # instead of blocking it.
                for lkb in range(NBLK):
                    kp = ypsum.tile([128, BLKW], F32, tag="y", name=f"kp1_{lkb}")
                    for kc in range(KC):
                        nc.tensor.matmul(
                            kp[:],
                            wk_sb[:, kc * GC + 128: kc * GC + 256],
                            xk_sb[kc][:, lkb * BLKW:(lkb + 1) * BLKW],
                            start=(kc == 0),
                            stop=(kc == KC - 1),
                        )
                    dst = kt_sb[1][:, lkb * BLKW:(lkb + 1) * BLKW]
                    if lkb % 2 == 0:
                        nc.vector.tensor_copy(dst, kp[:])
                    else:
                        nc.scalar.copy(dst, kp[:])

            # ---- main loop: attention + output projection ----
            ybar = [
                qkpool.tile([128, L], BF, tag=f"yb{oc}", name=f"yb{oc}")
                for oc in range(2)
            ]
            ptmaps = {}

            def emit_B(p):
                blk, hp = divmod(p, 2)
                lq0 = blk * BLKW
                units = [(hh, lk) for lk in range(LC) for hh in range(2)]
                ptloc = {}
                for u0 in range(0, len(units), 3):
                    grp = units[u0:u0 + 3]
                    w = len(grp) * BLKW
                    ps = spsum.tile([128, 3 * BLKW], F32, tag="s", name="pss")
                    for i, (hh, lk) in enumerate(grp):
                        h = 2 * hp + hh
                        oc = h // 2
                        r0 = (h % 2) * 64
                        nc.tensor.matmul(
                            ps[:, i * BLKW:(i + 1) * BLKW],
                            kt_sb[oc][r0:r0 + 64, lk * 128:(lk + 1) * 128],
                            qt_sb[oc][r0:r0 + 64, lq0:lq0 + BLKW],
                            start=True,
                            stop=True,
                        )
                    pt = ptpool.tile([128, 3 * BLKW], BF, tag="pt")
                    off = OFF_GROUPS_EVEN if p % 2 == 0 else OFF_GROUPS_ODD
                    if (u0 // 3) in off:
                        # DVE path: exp via Schraudolph bits, int16 written
                        # straight into the bf16 pt tile
                        nc.vector.tensor_scalar(
                            pt[:, 0:w].bitcast(I16),
                            ps[:, 0:w],
                            A_EXP,
                            B_EXP,
                            mybir.AluOpType.mult,
                            mybir.AluOpType.add,
                        )
                    else:
                        nc.scalar.activation(pt[:, 0:w], ps[:, 0:w], ExpF, scale=scale)
                    for i, (hh, lk) in enumerate(grp):
                        ptloc[(hh, lk)] = (pt, i * BLKW)
                ptmaps[p] = ptloc

            def emit_C(p):
                blk, hp = divmod(p, 2)
                lq0 = blk * BLKW
                ptloc = ptmaps.pop(p)
                yexts = [
                    ypsum.tile([128, BLKW], F32, tag="y", name="yext")
                    for _ in range(2)
                ]
                for lk in range(LC):
                    for hh in range(2):
                        h = 2 * hp + hh
                        pt, col = ptloc[(hh, lk)]
                        nc.tensor.matmul(
                            yexts[hh][0:D + 1, :],
                            v_sb[lk][:, h * (D + 1):(h + 1) * (D + 1)],
                            pt[:, col:col + BLKW],
                            start=(lk == 0),
                            stop=(lk == LC - 1),
                        )
                for hh in range(2):
                    h = 2 * hp + hh
                    yext = yexts[hh]
                    if debug_dump and p == 0 and hh == 0:
                        ydump = smpool.tile([D + 1, BLKW], F32, tag="ydump")
                        nc.vector.tensor_copy(ydump[:], yext[0:D + 1, :])
                        nc.sync.dma_start(dbg["yext"][:], ydump[:])
                    # rs copy to SBUF: reciprocal_approx_fast is a raw-bit
                    # DVE trick and must not read PSUM directly (HW-verified
                    # failure: denominators come out garbage)
                    rs = smpool.tile([1, BLKW], F32, tag="rs")
                    nc.vector.tensor_copy(rs[:], yext[D:D + 1, :])
                    rr = smpool.tile([1, BLKW], F32, tag="rr")
                    nc.vector.reciprocal_approx_fast(rr[:], rs[:])
                    rbc = smpool.tile([64, BLKW], F32, tag="rbc")
                    nc.gpsimd.partition_broadcast(rbc[:], rr[:])
                    if debug_dump and p == 0 and hh == 0:
                        nc.sync.dma_start(dbg["rbc"][:], rbc[:])
                    oc = h // 2
                    r0 = (h % 2) * 64
                    nc.vector.tensor_mul(
                        ybar[oc][r0:r0 + 64, lq0:lq0 + BLKW],
                        yext[0:D, :],
                        rbc[:],
                    )

            def emit_D(blk):
                for lc in range(4 * blk, 4 * blk + 4):
                    osb = opool.tile([128, C], F32, tag="osb")
                    for cc in range(2):
                        po = ypsum.tile([128, BLKW], F32, tag="y")
                        for kc2 in range(2):
                            nc.tensor.matmul(
                                po[:],
                                ybar[kc2][:, lc * 128:(lc + 1) * 128],
                                wo_sb[:, kc2 * C + cc * BLKW: kc2 * C + (cc + 1) * BLKW],
                                start=(kc2 == 0),
                                stop=(kc2 == 1),
                            )
                        nc.vector.tensor_copy(
                            osb[:, cc * BLKW:(cc + 1) * BLKW], po[:]
                        )
                    nc.sync.dma_start(out[lc * 128:(lc + 1) * 128, :], osb[:])

            def emit_D_tail(blk):
                # tail variant: the S-phase is over, so its 6 psum banks
                # are free — run the last output-projection blocks there
                # (keeping the y-ring for PV(7)+normalize) with drains
                # split across DVE and the now-idle ScalarE
                for lc in range(4 * blk, 4 * blk + 4):
                    osb = opool.tile([128, C], F32, tag="osb")
                    dps = spsum.tile([128, 3 * BLKW], F32, tag="s", name=f"dp{lc}")
                    for cc in range(2):
                        for kc2 in range(2):
                            nc.tensor.matmul(
                                dps[:, cc * BLKW:(cc + 1) * BLKW],
                                ybar[kc2][:, lc * 128:(lc + 1) * 128],
                                wo_sb[:, kc2 * C + cc * BLKW: kc2 * C + (cc + 1) * BLKW],
                                start=(kc2 == 0),
                                stop=(kc2 == 1),
                            )
                    nc.vector.tensor_copy(osb[:, 0:BLKW], dps[:, 0:BLKW])
                    nc.scalar.copy(osb[:, BLKW:2 * BLKW], dps[:, BLKW:2 * BLKW])
                    nc.sync.dma_start(out[lc * 128:(lc + 1) * 128, :], osb[:])

            emit_B(0)
            emit_K_oc1()
            emit_B(1)
            emit_V()   # V projection: PE filler under the first S/exp phases
            emit_C(0)
            for p in range(2, 8):
                if p % 2 == 0:
                    # qt block for this pair of S phases (data arrives via
                    # the column-sliced xq DMAs queued behind xv)
                    emit_q_blk(p // 2)
                emit_B(p)
                emit_C(p - 1)
                if p in (3, 5):
                    # D has a full block of slack: deprioritize it so it only
                    # fills PE gaps instead of preempting S^T production.
                    with tc.high_priority(offset=-400):
                        emit_D((p - 3) // 2)
            emit_C(7)
            emit_D_tail(2)
            emit_D_tail(3)

            if debug_dump:
                nc.sync.dma_start(dbg["qt0"][:], qt_sb[0][:])

    nc.compile()
    return nc


def _get_nc():
    if "nc" not in _CACHE:
        _CACHE["nc"] = _build()
    return _CACHE["nc"]


def _prep_in_maps(q, k, v, Wq, bq, Wk, bv_unused, Wv, Wo):
    del bv_unused
    xT = {}
    for b in range(B):
        xT[("q", b)] = np.ascontiguousarray(q[b].T).astype(BF16)
        xT[("k", b)] = np.ascontiguousarray(k[b].T).astype(BF16)
        xT[("v", b)] = np.ascontiguousarray(v[b].T).astype(BF16)
    in_maps = []
    for c in range(8):
        b, g = c // NG, c % NG
        wv_g = np.zeros((C, VW), dtype=BF16)
        for h in range(HPG):
            wv_g[:, h * (D + 1):h * (D + 1) + D] = Wv[
                :, g * GC + h * D: g * GC + (h + 1) * D
            ].astype(BF16)
        def tile_w(w):
            # [C_or_GC rows, n cols] -> [128, KC_chunks * n]: chunk kc holds
            # rows kc*128..kc*128+128 side by side
            r, n = w.shape
            return np.ascontiguousarray(
                w.reshape(r // 128, 128, n).transpose(1, 0, 2).reshape(128, -1)
            )
        in_maps.append({
            "xqT": xT[("q", b)],
            "xkT": xT[("k", b)],
            "xvT": xT[("v", b)],
            "wq": tile_w(Wq[:, g * GC:(g + 1) * GC].astype(BF16)),
            "wk": tile_w(Wk[:, g * GC:(g + 1) * GC].astype(BF16)),
            "wv": tile_w(wv_g),
            "wo": tile_w(Wo[g * GC:(g + 1) * GC, :].astype(BF16)),
            "bq": bq[g * GC:(g + 1) * GC].reshape(GC, 1).astype(np.float32),
        })
    return in_maps


def kernel(q, k, v, Wq, bq, Wk, bk, Wv, bv, Wo, bo):
    q = np.asarray(q, dtype=np.float32)
    k = np.asarray(k, dtype=np.float32)
    v = np.asarray(v, dtype=np.float32)
    Wq = np.asarray(Wq, dtype=np.float32)
    bq = np.asarray(bq, dtype=np.float32)
    Wk = np.asarray(Wk, dtype=np.float32)
    Wv = np.asarray(Wv, dtype=np.float32)
    bv = np.asarray(bv, dtype=np.float32)
    Wo = np.asarray(Wo, dtype=np.float32)
    bo = np.asarray(bo, dtype=np.float32)

    nc = _get_nc()
    in_maps = _prep_in_maps(q, k, v, Wq, bq, Wk, None, Wv, Wo)

    trace = bool(int(os.environ.get("BASS_KERNEL_PROFILE", "0")))
    kwargs = {}
    if trace:
        try:
            from ntff_hook import install as _install_hook
            _install_hook()
        except Exception:
            pass
        kwargs = {"trace": True}
        td = os.environ.get("BASS_KERNEL_TRACE_DIR")
        if td:
            kwargs["tmpdir"] = td
    res = run_bass_kernel_spmd(nc, in_maps, core_ids=list(range(8)), **kwargs)
    _CACHE["last_exec_time_ns"] = res.exec_time_ns

    # host gather: sum group partials per batch, add folded bias
    cvec = (bv.astype(np.float64) @ Wo.astype(np.float64) + bo).astype(np.float32)
    full = np.empty((B, L, C), dtype=np.float32)
    for b in range(B):
        acc = res.results[b * NG]["out"].astype(np.float32)
        for g in range(1, NG):
            acc = acc + res.results[b * NG + g]["out"]
        full[b] = acc + cvec[None, :]
    return full


# revision 33
# speedup vs baseline: 1.1553x; 1.0220x over previous
"""Multi-head attention (B=2, L=2048, C=1024, H=16, D=64) on 8 trn2 NeuronCores.

Sharding: core c -> (batch b = c//4, head-group g = c%4, 4 heads per group).
Tensor-parallel over heads: W_q/W_k/W_v column-sliced per group, W_o
row-sliced; each core returns a partial output projection for its batch and
the host sums the 4 group partials (the all-reduce of the hint, done on host
since the kernel contract is full-in/full-out).

Per-core device pipeline (all matmuls bf16 with fp32 PSUM accumulation):
  A) Kt = (k @ Wk)^T first (full-width accumulation; S needs all of K),
     then Qt per 512-query block with column-sliced xq DMAs so the
     attention phase starts as soon as kt + qt-block0 are ready (~26us)
     instead of after both full projections (~50us). V in natural
     [L, channels] layout with 4 interleaved ones-columns ([V_h | 1] per
     head) for the softmax row-sum trick.
  B) S^T tiles (Lk on partitions) per (lq-block, head); exp split between
     ScalarE (native Exp activation) and DVE (Schraudolph int16 bit-trick:
     bf16 bits of exp(x) ~ int16(x*128*log2e/8 + 16250.6), max rel err
     ~3.3% which washes out through the softmax normalization) so the two
     engines share the ~109us/core exp workload.
  C) Yext^T = [V_h | 1]^T @ P^T  (M=65: row 64 = softmax denominator),
     then row-normalize via DVE reciprocal (read straight from PSUM) +
     GPSIMD partition-broadcast.
  D) out_partial = Ybar @ Wo_g, fp32, DMA'd out.

b_k is dropped (adds a per-query constant to logits -> softmax invariant);
b_v is folded into the host-side constant (b_v @ W_o + b_o) since sum(P)=1.
"""

import os
import numpy as np
import ml_dtypes

import concourse.bass as bass
import concourse.mybir as mybir
from concourse import bacc
from concourse.tile import TileContext
from concourse.bass_utils import run_bass_kernel_spmd

BF16 = ml_dtypes.bfloat16
BF = mybir.dt.bfloat16
F32 = mybir.dt.float32
I16 = mybir.dt.int16

B, L, C = 2, 2048, 1024
H, D = 16, 64
NG = 4            # head-groups (cores per batch)
HPG = 4           # heads per group
GC = HPG * D      # 256 channels per group
KC = C // 128     # 8 contraction chunks for projections
LC = L // 128     # 16 L-chunks
NBLK = 4          # lq blocks of 512
BLKW = 512
VW = HPG * (D + 1)  # 260: interleaved [V_h | ones] columns

# Schraudolph exp-in-bf16-bits: bits16 = round(x*(128*log2e)*scale + B)
# with scale = 1/sqrt(D) = 1/8 folded in. B calibrated for round-to-nearest
# (hardware-verified semantics), max rel err 3.3%, mean +1.1%.
A_EXP = 128.0 / float(np.log(2.0)) / 8.0
B_EXP = 16250.6
# groups of each lq-block x head-pair handed to DVE instead of ScalarE
# (HW-measured: DVE exp runs ~1.14x the ScalarE time per column; 4 of 16
# groups balances both engines including their other work)
OFF_GROUPS = (1, 5, 9, 13)

_CACHE = {}


def _build(debug_dump=False):
    nc = bacc.Bacc("TRN2", target_bir_lowering=False, debug=False, num_devices=8)
    dbg = {}
    if debug_dump:
        dbg["qt0"] = nc.declare_dram_parameter("d_qt0", [128, L], BF, isOutput=True)
        dbg["kt0"] = nc.declare_dram_parameter("d_kt0", [128, L], BF, isOutput=True)
        dbg["v0"] = nc.declare_dram_parameter("d_v0", [128, VW], BF, isOutput=True)
        dbg["pt"] = nc.declare_dram_parameter("d_pt", [128, 3 * BLKW], BF, isOutput=True)
        dbg["yext"] = nc.declare_dram_parameter("d_yext", [D + 1, BLKW], F32, isOutput=True)
        dbg["rbc"] = nc.declare_dram_parameter("d_rbc", [64, BLKW], F32, isOutput=True)

    xq = nc.declare_dram_parameter("xqT", [C, L], BF, isOutput=False)
    xk = nc.declare_dram_parameter("xkT", [C, L], BF, isOutput=False)
    xv = nc.declare_dram_parameter("xvT", [C, L], BF, isOutput=False)
    wq = nc.declare_dram_parameter("wq", [128, KC * GC], BF, isOutput=False)
    wk = nc.declare_dram_parameter("wk", [128, KC * GC], BF, isOutput=False)
    wv = nc.declare_dram_parameter("wv", [128, KC * VW], BF, isOutput=False)
    wo = nc.declare_dram_parameter("wo", [128, 2 * C], BF, isOutput=False)
    bq = nc.declare_dram_parameter("bq", [GC, 1], F32, isOutput=False)
    out = nc.declare_dram_parameter("out", [L, C], BF, isOutput=True)

    with TileContext(nc) as tc:
        with (
            tc.tile_pool(name="w", bufs=1) as wpool,
            tc.tile_pool(name="xb", bufs=1) as xbpool,
            tc.tile_pool(name="xvs", bufs=2) as xvspool,
            tc.tile_pool(name="qx", bufs=2) as qxpool,
            tc.tile_pool(name="qk", bufs=1) as qkpool,
            tc.tile_pool(name="vsb", bufs=1) as vpool,
            tc.tile_pool(name="pt", bufs=34) as ptpool,
            tc.tile_pool(name="sm", bufs=2) as smpool,
            tc.tile_pool(name="ob", bufs=4) as opool,
            tc.tile_pool(name="ps", bufs=3, space="PSUM") as spsum,
            tc.tile_pool(name="py", bufs=2, space="PSUM") as ypsum,
        ):
            Ident = mybir.ActivationFunctionType.Identity
            ExpF = mybir.ActivationFunctionType.Exp
            scale = 1.0 / float(np.sqrt(D))

            # ---- weights / constants (gpsimd DMA queue; sync queue is
            # reserved for the big x streams in priority order) ----
            wk_sb = wpool.tile([128, KC * GC], BF, tag="wk")
            nc.sync.dma_start(wk_sb[:], wk[:])
            wq_sb = wpool.tile([128, KC * GC], BF, tag="wq")
            nc.sync.dma_start(wq_sb[:], wq[:])
            bq_sb = []
            for oc in range(2):
                t = wpool.tile([128, 1], F32, tag=f"bq{oc}")
                nc.gpsimd.dma_start(t[:], bq[oc * 128:(oc + 1) * 128, :])
                bq_sb.append(t)
            wv_sb = wpool.tile([128, KC * VW], BF, tag="wv")
            wo_sb = wpool.tile([128, 2 * C], BF, tag="wo")
            # ones-pattern: +1.0 at the interleaved ones-columns (65h+64)
            vpat = wpool.tile([128, VW], F32, tag="vpat")
            nc.vector.memset(vpat[:], 0.0)
            for h in range(HPG):
                nc.vector.memset(vpat[:, h * (D + 1) + D: h * (D + 1) + D + 1], 1.0)

            # The sync queue uses hardware DGE (~200GB/s); the gpsimd queue
            # is software-DGE and only manages ~60-100GB/s. Split the big x
            # streams ~3:1 so both queues finish together.
            def dma2(i, dst, src):
                eng = nc.gpsimd if i % 4 == 3 else nc.sync
                eng.dma_start(dst, src)

            # ---- K projection ----
            # oc0 accumulates FULL-width while the xk chunks stream in (its
            # matmuls trail the chunk arrivals), so kt-oc0 is ready right
            # after the xk DMA finishes. S(p0) = heads 0,1 only needs
            # kt-oc0 + qt-blk0, so the attention phase starts ~23us.
            # oc1 (only needed by S(p1) ~8us later) re-reads the resident
            # chunks column-blocked through the 2-bank y-ring, leaving the
            # spsum banks free for Q-blk0 and the first S groups.
            kt_sb = [
                qkpool.tile([128, L], BF, tag=f"kt{oc}", name=f"kt{oc}")
                for oc in range(2)
            ]
            xk_sb = []
            for kc in range(KC):
                t = xbpool.tile([128, L], BF, tag=f"xb{kc}", name="xkt")
                dma2(kc, t[:], xk[kc * 128:(kc + 1) * 128, :])
                xk_sb.append(t)
            ps_a = [spsum.tile([128, 2 * BLKW], F32, tag="s", name=f"kpa{i}") for i in range(2)]
            for kc in range(KC):
                lhs = wk_sb[:, kc * GC: kc * GC + 128]
                for lqb in range(NBLK):
                    nc.tensor.matmul(
                        ps_a[lqb // 2][:, (lqb % 2) * BLKW:(lqb % 2 + 1) * BLKW],
                        lhs,
                        xk_sb[kc][:, lqb * BLKW:(lqb + 1) * BLKW],
                        start=(kc == 0),
                        stop=(kc == KC - 1),
                    )
            nc.vector.tensor_copy(kt_sb[0][:, 0:2 * BLKW], ps_a[0][:])
            nc.scalar.activation(kt_sb[0][:, 2 * BLKW:L], ps_a[1][:], Ident)

            # ---- Q projection, per 512-col lq-block (column-sliced DMA,
            # one 3-bank spsum tile per block: oc0 cols 0:512, oc1 512:1024)
            qt_sb = [
                qkpool.tile([128, L], BF, tag=f"qt{oc}", name=f"qt{oc}")
                for oc in range(2)
            ]
            qslices = {}

            def emit_q_dma(blk):
                tiles = []
                for kc in range(KC):
                    xt = qxpool.tile([128, BLKW], BF, tag=f"qx{kc % 4}")
                    dma2(kc, xt[:],
                         xq[kc * 128:(kc + 1) * 128, blk * BLKW:(blk + 1) * BLKW])
                    tiles.append(xt)
                qslices[blk] = tiles

            def emit_q_blk(blk):
                qps = spsum.tile([128, 2 * BLKW], F32, tag="s", name=f"qps{blk}")
                tiles = qslices.pop(blk)
                for kc in range(KC):
                    for oc in range(2):
                        nc.tensor.matmul(
                            qps[:, oc * BLKW:(oc + 1) * BLKW],
                            wq_sb[:, kc * GC + oc * 128: kc * GC + (oc + 1) * 128],
                            tiles[kc][:],
                            start=(kc == 0),
                            stop=(kc == KC - 1),
                        )
                dst0 = qt_sb[0][:, blk * BLKW:(blk + 1) * BLKW]
                dst1 = qt_sb[1][:, blk * BLKW:(blk + 1) * BLKW]
                nc.vector.tensor_scalar_add(dst0, qps[:, 0:BLKW], bq_sb[0][:])
                nc.scalar.activation(
                    dst1, qps[:, BLKW:2 * BLKW], Ident, bias=bq_sb[1][:]
                )

            emit_q_dma(0)
            emit_q_blk(0)

            if debug_dump:
                nc.sync.dma_start(dbg["kt0"][:], kt_sb[0][:])

            # V weights on the slow queue ahead of its xvs share (needed
            # by ~26us); W_o rides later
            nc.gpsimd.dma_start(wv_sb[:], wv[:])

            # ---- xv loads as 512-wide column slices into their own small
            # pool: V-proj unit lc only needs slice column-block lc//4 of
            # every channel chunk, so the V projection streams right behind
            # these DMAs instead of gating on full-chunk residency ----
            xvs = {}
            for vb in range(NBLK):
                for kc in range(KC):
                    t = xvspool.tile([128, BLKW], BF, tag=f"xvs{kc}", name="xvst")
                    dma2(kc, t[:],
                         xv[kc * 128:(kc + 1) * 128, vb * BLKW:(vb + 1) * BLKW])
                    xvs[(kc, vb)] = t
            # q block-1 slices behind xv on the qx tags; blocks 2-3 reuse
            # the xvs tags (free once the V projection consumes them)
            emit_q_dma(1)
            for blk in (2, 3):
                tiles = []
                for kc in range(KC):
                    t = xvspool.tile([128, BLKW], BF, tag=f"xvs{kc}", name="qxl")
                    dma2(kc, t[:],
                         xq[kc * 128:(kc + 1) * 128, blk * BLKW:(blk + 1) * BLKW])
                    tiles.append(t)
                qslices[blk] = tiles

            nc.gpsimd.dma_start(wo_sb[:], wo[:])

            # dummy partition_broadcast: forces the GPSIMD custom-op
            # library load (~8us) to happen here, overlapped with the DMA
            # phase, instead of stalling the first softmax normalize
            dmb_i = wpool.tile([1, 8], F32, tag="dmb_i")
            nc.vector.memset(dmb_i[:], 0.0)
            dmb_o = wpool.tile([2, 8], F32, tag="dmb_o")
            nc.gpsimd.partition_broadcast(dmb_o[:], dmb_i[:])

            # ---- V projection (emitted inside main loop as filler) ----
            v_sb = []

            def emit_V():
                for lc in range(LC):
                    vps = ypsum.tile([128, BLKW], F32, tag="y", name="vps")
                    for kc in range(KC):
                        nc.tensor.matmul(
                            vps[:, 0:VW],
                            xvs[(kc, lc // 4)][:, (lc % 4) * 128:(lc % 4 + 1) * 128],
                            wv_sb[:, kc * VW:(kc + 1) * VW],
                            start=(kc == 0),
                            stop=(kc == KC - 1),
                        )
                    vt = vpool.tile([128, VW], BF, tag=f"v{lc}", name="vt")
                    nc.vector.tensor_add(vt[:], vps[:, 0:VW], vpat[:])
                    v_sb.append(vt)
                if debug_dump:
                    nc.sync.dma_start(dbg["v0"][:], v_sb[0][:])

            def emit_K_oc1():
                # K-oc1 column-blocked on the y-ring; chunks already
                # resident. Emitted AFTER B(0) so the scheduler slots these
                # matmuls into the exp-paced gaps of the first S phase
                # instead of blocking it.
                for lkb in range(NBLK):
                    kp = ypsum.tile([128, BLKW], F32, tag="y", name=f"kp1_{lkb}")
                    for kc in range(KC):
                        nc.tensor.matmul(
                            kp[:],
                            wk_sb[:, kc * GC + 128: kc * GC + 256],
                            xk_sb[kc][:, lkb * BLKW:(lkb + 1) * BLKW],
                            start=(kc == 0),
                            stop=(kc == KC - 1),
                        )
                    dst = kt_sb[1][:, lkb * BLKW:(lkb + 1) * BLKW]
                    if lkb % 2 == 0:
                        nc.vector.tensor_copy(dst, kp[:])
                    else:
                        nc.scalar.copy(dst, kp[:])

            # ---- main loop: attention + output projection ----
            ybar = [
                qkpool.tile([128, L], BF, tag=f"yb{oc}", name=f"yb{oc}")
                for oc in range(2)
            ]
            ptmaps = {}

            def emit_B(p):
                blk, hp = divmod(p, 2)
                lq0 = blk * BLKW
                units = [(hh, lk) for lk in range(LC) for hh in range(2)]
                ptloc = {}
                for u0 in range(0, len(units), 2):
                    grp = units[u0:u0 + 2]
                    w = len(grp) * BLKW
                    ps = spsum.tile([128, 2 * BLKW], F32, tag="s", name="pss")
                    for i, (hh, lk) in enumerate(grp):
                        h = 2 * hp + hh
                        oc = h // 2
                        r0 = (h % 2) * 64
                        nc.tensor.matmul(
                            ps[:, i * BLKW:(i + 1) * BLKW],
                            kt_sb[oc][r0:r0 + 64, lk * 128:(lk + 1) * 128],
                            qt_sb[oc][r0:r0 + 64, lq0:lq0 + BLKW],
                            start=True,
                            stop=True,
                        )
                    pt = ptpool.tile([128, 2 * BLKW], BF, tag="pt")
                    if (u0 // 2) in OFF_GROUPS:
                        # DVE path: exp via Schraudolph bits, int16 written
                        # straight into the bf16 pt tile
                        nc.vector.tensor_scalar(
                            pt[:, 0:w].bitcast(I16),
                            ps[:, 0:w],
                            A_EXP,
                            B_EXP,
                            mybir.AluOpType.mult,
                            mybir.AluOpType.add,
                        )
                    else:
                        nc.scalar.activation(pt[:, 0:w], ps[:, 0:w], ExpF, scale=scale)
                    for i, (hh, lk) in enumerate(grp):
                        ptloc[(hh, lk)] = (pt, i * BLKW)
                ptmaps[p] = ptloc

            def emit_C(p):
                blk, hp = divmod(p, 2)
                lq0 = blk * BLKW
                ptloc = ptmaps.pop(p)
                yexts = [
                    ypsum.tile([128, BLKW], F32, tag="y", name="yext")
                    for _ in range(2)
                ]
                for lk in range(LC):
                    for hh in range(2):
                        h = 2 * hp + hh
                        pt, col = ptloc[(hh, lk)]
                        nc.tensor.matmul(
                            yexts[hh][0:D + 1, :],
                            v_sb[lk][:, h * (D + 1):(h + 1) * (D + 1)],
                            pt[:, col:col + BLKW],
                            start=(lk == 0),
                            stop=(lk == LC - 1),
                        )
                for hh in range(2):
                    h = 2 * hp + hh
                    yext = yexts[hh]
                    if debug_dump and p == 0 and hh == 0:
                        ydump = smpool.tile([D + 1, BLKW], F32, tag="ydump")
                        nc.vector.tensor_copy(ydump[:], yext[0:D + 1, :])
                        nc.sync.dma_start(dbg["yext"][:], ydump[:])
                    # rs copy to SBUF: reciprocal_approx_fast is a raw-bit
                    # DVE trick and must not read PSUM directly (HW-verified
                    # failure: denominators come out garbage)
                    rs = smpool.tile([1, BLKW], F32, tag="rs")
                    nc.vector.tensor_copy(rs[:], yext[D:D + 1, :])
                    rr = smpool.tile([1, BLKW], F32, tag="rr")
                    nc.vector.reciprocal_approx_fast(rr[:], rs[:])
                    rbc = smpool.tile([64, BLKW], F32, tag="rbc")
                    nc.gpsimd.partition_broadcast(rbc[:], rr[:])
                    if debug_dump and p == 0 and hh == 0:
                        nc.sync.dma_start(dbg["rbc"][:], rbc[:])
                    oc = h // 2
                    r0 = (h % 2) * 64
                    nc.vector.tensor_mul(
                        ybar[oc][r0:r0 + 64, lq0:lq0 + BLKW],
                        yext[0:D, :],
                        rbc[:],
                    )

            def emit_D(blk):
                for lc in range(4 * blk, 4 * blk + 4):
                    osb = opool.tile([128, C], BF, tag="osb")
                    for cc in range(2):
                        po = ypsum.tile([128, BLKW], F32, tag="y")
                        for kc2 in range(2):
                            nc.tensor.matmul(
                                po[:],
                                ybar[kc2][:, lc * 128:(lc + 1) * 128],
                                wo_sb[:, kc2 * C + cc * BLKW: kc2 * C + (cc + 1) * BLKW],
                                start=(kc2 == 0),
                                stop=(kc2 == 1),
                            )
                        nc.vector.tensor_copy(
                            osb[:, cc * BLKW:(cc + 1) * BLKW], po[:]
                        )
                    nc.sync.dma_start(out[lc * 128:(lc + 1) * 128, :], osb[:])

            def emit_D_tail(blk):
                # tail variant: the S-phase is over, so its 6 psum banks
                # are free — run the last output-projection blocks there
                # (keeping the y-ring for PV(7)+normalize) with drains
                # split across DVE and the now-idle ScalarE
                for lc in range(4 * blk, 4 * blk + 4):
                    osb = opool.tile([128, C], BF, tag="osb")
                    dps = spsum.tile([128, 2 * BLKW], F32, tag="s", name=f"dp{lc}")
                    for cc in range(2):
                        for kc2 in range(2):
                            nc.tensor.matmul(
                                dps[:, cc * BLKW:(cc + 1) * BLKW],
                                ybar[kc2][:, lc * 128:(lc + 1) * 128],
                                wo_sb[:, kc2 * C + cc * BLKW: kc2 * C + (cc + 1) * BLKW],
                                start=(kc2 == 0),
                                stop=(kc2 == 1),
                            )
                    nc.vector.tensor_copy(osb[:, 0:BLKW], dps[:, 0:BLKW])
                    nc.scalar.copy(osb[:, BLKW:2 * BLKW], dps[:, BLKW:2 * BLKW])
                    nc.sync.dma_start(out[lc * 128:(lc + 1) * 128, :], osb[:])

            emit_B(0)
            emit_K_oc1()
            emit_B(1)
            emit_V()   # V projection: PE filler under the first S/exp phases
            emit_C(0)
            for p in range(2, 8):
                if p % 2 == 0:
                    # qt block for this pair of S phases (data arrives via
                    # the column-sliced xq DMAs queued behind xv)
                    emit_q_blk(p // 2)
                emit_B(p)
                emit_C(p - 1)
                if p in (3, 5):
                    # D has a full block of slack: deprioritize it so it only
                    # fills PE gaps instead of preempting S^T production.
                    with tc.high_priority(offset=-400):
                        emit_D((p - 3) // 2)
            emit_C(7)
            emit_D_tail(2)
            emit_D_tail(3)

            if debug_dump:
                nc.sync.dma_start(dbg["qt0"][:], qt_sb[0][:])

    nc.compile()
    return nc


def _get_nc():
    if "nc" not in _CACHE:
        _CACHE["nc"] = _build()
    return _CACHE["nc"]


def _prep_in_maps(q, k, v, Wq, bq, Wk, bv_unused, Wv, Wo):
    del bv_unused
    xT = {}
    for b in range(B):
        xT[("q", b)] = np.ascontiguousarray(q[b].T).astype(BF16)
        xT[("k", b)] = np.ascontiguousarray(k[b].T).astype(BF16)
        xT[("v", b)] = np.ascontiguousarray(v[b].T).astype(BF16)
    in_maps = []
    for c in range(8):
        b, g = c // NG, c % NG
        wv_g = np.zeros((C, VW), dtype=BF16)
        for h in range(HPG):
            wv_g[:, h * (D + 1):h * (D + 1) + D] = Wv[
                :, g * GC + h * D: g * GC + (h + 1) * D
            ].astype(BF16)
        def tile_w(w):
            # [C_or_GC rows, n cols] -> [128, KC_chunks * n]: chunk kc holds
            # rows kc*128..kc*128+128 side by side
            r, n = w.shape
            return np.ascontiguousarray(
                w.reshape(r // 128, 128, n).transpose(1, 0, 2).reshape(128, -1)
            )
        in_maps.append({
            "xqT": xT[("q", b)],
            "xkT": xT[("k", b)],
            "xvT": xT[("v", b)],
            "wq": tile_w(Wq[:, g * GC:(g + 1) * GC].astype(BF16)),
            "wk": tile_w(Wk[:, g * GC:(g + 1) * GC].astype(BF16)),
            "wv": tile_w(wv_g),
            "wo": tile_w(Wo[g * GC:(g + 1) * GC, :].astype(BF16)),
            "bq": bq[g * GC:(g + 1) * GC].reshape(GC, 1).astype(np.float32),
        })
    return in_maps


def kernel(q, k, v, Wq, bq, Wk, bk, Wv, bv, Wo, bo):
    q = np.asarray(q, dtype=np.float32)
    k = np.asarray(k, dtype=np.float32)
    v = np.asarray(v, dtype=np.float32)
    Wq = np.asarray(Wq, dtype=np.float32)
    bq = np.asarray(bq, dtype=np.float32)
    Wk = np.asarray(Wk, dtype=np.float32)
    Wv = np.asarray(Wv, dtype=np.float32)
    bv = np.asarray(bv, dtype=np.float32)
    Wo = np.asarray(Wo, dtype=np.float32)
    bo = np.asarray(bo, dtype=np.float32)

    nc = _get_nc()
    in_maps = _prep_in_maps(q, k, v, Wq, bq, Wk, None, Wv, Wo)

    trace = bool(int(os.environ.get("BASS_KERNEL_PROFILE", "0")))
    kwargs = {}
    if trace:
        try:
            from ntff_hook import install as _install_hook
            _install_hook()
        except Exception:
            pass
        kwargs = {"trace": True}
        td = os.environ.get("BASS_KERNEL_TRACE_DIR")
        if td:
            kwargs["tmpdir"] = td
    res = run_bass_kernel_spmd(nc, in_maps, core_ids=list(range(8)), **kwargs)
    _CACHE["last_exec_time_ns"] = res.exec_time_ns

    # host gather: sum group partials per batch, add folded bias
    cvec = (bv.astype(np.float64) @ Wo.astype(np.float64) + bo).astype(np.float32)
    full = np.empty((B, L, C), dtype=np.float32)
    for b in range(B):
        acc = res.results[b * NG]["out"].astype(np.float32)
        for g in range(1, NG):
            acc = acc + res.results[b * NG + g]["out"].astype(np.float32)
        full[b] = acc + cvec[None, :]
    return full


# revision 43
# speedup vs baseline: 1.1773x; 1.0190x over previous
"""Multi-head attention (B=2, L=2048, C=1024, H=16, D=64) on 8 trn2 NeuronCores.

Sharding: core c -> (batch b = c//4, head-group g = c%4, 4 heads per group).
Tensor-parallel over heads: W_q/W_k/W_v column-sliced per group, W_o
row-sliced; each core returns a partial output projection for its batch and
the host sums the 4 group partials (the all-reduce of the hint, done on host
since the kernel contract is full-in/full-out).

Per-core device pipeline (all matmuls bf16 with fp32 PSUM accumulation):
  A) Kt = (k @ Wk)^T first (full-width accumulation; S needs all of K),
     then Qt per 512-query block with column-sliced xq DMAs so the
     attention phase starts as soon as kt + qt-block0 are ready (~26us)
     instead of after both full projections (~50us). V in natural
     [L, channels] layout with 4 interleaved ones-columns ([V_h | 1] per
     head) for the softmax row-sum trick.
  B) S^T tiles (Lk on partitions) per (lq-block, head); exp split between
     ScalarE (native Exp activation) and DVE (Schraudolph int16 bit-trick:
     bf16 bits of exp(x) ~ int16(x*128*log2e/8 + 16250.6), max rel err
     ~3.3% which washes out through the softmax normalization) so the two
     engines share the ~109us/core exp workload.
  C) Yext^T = [V_h | 1]^T @ P^T  (M=65: row 64 = softmax denominator),
     then row-normalize via DVE reciprocal (read straight from PSUM) +
     GPSIMD partition-broadcast.
  D) out_partial = Ybar @ Wo_g, fp32, DMA'd out.

b_k is dropped (adds a per-query constant to logits -> softmax invariant);
b_v is folded into the host-side constant (b_v @ W_o + b_o) since sum(P)=1.
"""

import os
import numpy as np
import ml_dtypes

import concourse.bass as bass
import concourse.mybir as mybir
from concourse import bacc
from concourse.tile import TileContext
from concourse.bass_utils import run_bass_kernel_spmd

BF16 = ml_dtypes.bfloat16
BF = mybir.dt.bfloat16
F32 = mybir.dt.float32
I16 = mybir.dt.int16

B, L, C = 2, 2048, 1024
H, D = 16, 64
NG = 4            # head-groups (cores per batch)
HPG = 4           # heads per group
GC = HPG * D      # 256 channels per group
KC = C // 128     # 8 contraction chunks for projections
LC = L // 128     # 16 L-chunks
NBLK = 4          # lq blocks of 512
BLKW = 512
VW = HPG * (D + 1)  # 260: interleaved [V_h | ones] columns

# Schraudolph exp-in-bf16-bits: bits16 = round(x*(128*log2e)*scale + B)
# with scale = 1/sqrt(D) = 1/8 folded in. B calibrated for round-to-nearest
# (hardware-verified semantics), max rel err 3.3%, mean +1.1%.
A_EXP = 128.0 / float(np.log(2.0)) / 8.0
B_EXP = 16250.6
# groups of each lq-block x head-pair handed to DVE instead of ScalarE
# (HW-measured: DVE exp runs ~1.14x the ScalarE time per column; 4 of 16
# groups balances both engines including their other work). DVE groups
# get their own 1-buffer psum ring ("sd") so a DVE exp queued behind
# other DVE work never stalls the ScalarE group rotation.
OFF_GROUPS = (2, 6, 10, 14)

_CACHE = {}


def _build(debug_dump=False):
    nc = bacc.Bacc("TRN2", target_bir_lowering=False, debug=False, num_devices=8)
    dbg = {}
    if debug_dump:
        dbg["qt0"] = nc.declare_dram_parameter("d_qt0", [128, L], BF, isOutput=True)
        dbg["kt0"] = nc.declare_dram_parameter("d_kt0", [128, L], BF, isOutput=True)
        dbg["v0"] = nc.declare_dram_parameter("d_v0", [128, VW], BF, isOutput=True)
        dbg["pt"] = nc.declare_dram_parameter("d_pt", [128, 3 * BLKW], BF, isOutput=True)
        dbg["yext"] = nc.declare_dram_parameter("d_yext", [D + 1, BLKW], F32, isOutput=True)
        dbg["rbc"] = nc.declare_dram_parameter("d_rbc", [64, BLKW], F32, isOutput=True)

    xq = nc.declare_dram_parameter("xqT", [C, L], BF, isOutput=False)
    xk = nc.declare_dram_parameter("xkT", [C, L], BF, isOutput=False)
    xv = nc.declare_dram_parameter("xvT", [C, L], BF, isOutput=False)
    wq = nc.declare_dram_parameter("wq", [128, KC * GC], BF, isOutput=False)
    wk = nc.declare_dram_parameter("wk", [128, KC * GC], BF, isOutput=False)
    wv = nc.declare_dram_parameter("wv", [128, KC * VW], BF, isOutput=False)
    wo = nc.declare_dram_parameter("wo", [128, 2 * C], BF, isOutput=False)
    bq = nc.declare_dram_parameter("bq", [GC, 1], F32, isOutput=False)
    out = nc.declare_dram_parameter("out", [L, C], BF, isOutput=True)

    with TileContext(nc) as tc:
        with (
            tc.tile_pool(name="w", bufs=1) as wpool,
            tc.tile_pool(name="xb", bufs=1) as xbpool,
            tc.tile_pool(name="xvs", bufs=2) as xvspool,
            tc.tile_pool(name="qx", bufs=2) as qxpool,
            tc.tile_pool(name="qk", bufs=1) as qkpool,
            tc.tile_pool(name="vsb", bufs=1) as vpool,
            tc.tile_pool(name="pt", bufs=40) as ptpool,
            tc.tile_pool(name="sm", bufs=2) as smpool,
            tc.tile_pool(name="ob", bufs=3) as opool,
            tc.tile_pool(name="ps", bufs=2, space="PSUM") as spsum,
            tc.tile_pool(name="py", bufs=2, space="PSUM") as ypsum,
        ):
            Ident = mybir.ActivationFunctionType.Identity
            ExpF = mybir.ActivationFunctionType.Exp
            scale = 1.0 / float(np.sqrt(D))

            # ---- weights / constants (gpsimd DMA queue; sync queue is
            # reserved for the big x streams in priority order) ----
            wk_sb = wpool.tile([128, KC * GC], BF, tag="wk")
            nc.sync.dma_start(wk_sb[:], wk[:])
            wq_sb = wpool.tile([128, KC * GC], BF, tag="wq")
            nc.gpsimd.dma_start(wq_sb[:], wq[:])
            bq_sb = []
            for oc in range(2):
                t = wpool.tile([128, 1], F32, tag=f"bq{oc}")
                nc.gpsimd.dma_start(t[:], bq[oc * 128:(oc + 1) * 128, :])
                bq_sb.append(t)
            wv_sb = wpool.tile([128, KC * VW], BF, tag="wv")
            wo_sb = wpool.tile([128, 2 * C], BF, tag="wo")
            # ones-pattern: +1.0 at the interleaved ones-columns (65h+64)
            vpat = wpool.tile([128, VW], F32, tag="vpat")
            nc.vector.memset(vpat[:], 0.0)
            for h in range(HPG):
                nc.vector.memset(vpat[:, h * (D + 1) + D: h * (D + 1) + D + 1], 1.0)

            # The sync queue uses hardware DGE (~200GB/s); the gpsimd queue
            # is software-DGE and only manages ~60-100GB/s. Split the big x
            # streams ~3:1 so both queues finish together.
            def dma2(i, dst, src):
                eng = nc.gpsimd if i % 4 == 3 else nc.sync
                eng.dma_start(dst, src)

            # ---- K projection ----
            # oc0 accumulates FULL-width while the xk chunks stream in (its
            # matmuls trail the chunk arrivals), so kt-oc0 is ready right
            # after the xk DMA finishes. S(p0) = heads 0,1 only needs
            # kt-oc0 + qt-blk0, so the attention phase starts ~23us.
            # oc1 (only needed by S(p1) ~8us later) re-reads the resident
            # chunks column-blocked through the 2-bank y-ring, leaving the
            # spsum banks free for Q-blk0 and the first S groups.
            kt_sb = [
                qkpool.tile([128, L], BF, tag=f"kt{oc}", name=f"kt{oc}")
                for oc in range(2)
            ]
            # xk as 16 resident half-chunks [128,1024]; the two earliest
            # channel chunks ride the slow gpsimd queue (the K chain
            # tolerates early chunks arriving first), the rest stream on
            # sync behind wk
            xk_sb = {}
            for kc in range(KC):
                eng = nc.gpsimd if kc < 2 else nc.sync
                for hf in range(2):
                    t = xbpool.tile([128, 2 * BLKW], BF, tag=f"xh{kc}_{hf}", name="xkt")
                    eng.dma_start(
                        t[:],
                        xk[kc * 128:(kc + 1) * 128, hf * 2 * BLKW:(hf + 1) * 2 * BLKW],
                    )
                    xk_sb[(kc, hf)] = t
            ps_a = [spsum.tile([128, 2 * BLKW], F32, tag="s", name=f"kpa{i}") for i in range(2)]
            for kc in range(KC):
                lhs = wk_sb[:, kc * GC: kc * GC + 128]
                for lqb in range(NBLK):
                    nc.tensor.matmul(
                        ps_a[lqb // 2][:, (lqb % 2) * BLKW:(lqb % 2 + 1) * BLKW],
                        lhs,
                        xk_sb[(kc, lqb // 2)][:, (lqb % 2) * BLKW:(lqb % 2 + 1) * BLKW],
                        start=(kc == 0),
                        stop=(kc == KC - 1),
                    )
            nc.vector.tensor_copy(kt_sb[0][:, 0:2 * BLKW], ps_a[0][:])
            nc.scalar.activation(kt_sb[0][:, 2 * BLKW:L], ps_a[1][:], Ident)

            # ---- Q projection, per 512-col lq-block (column-sliced DMA,
            # one 3-bank spsum tile per block: oc0 cols 0:512, oc1 512:1024)
            qt_sb = [
                qkpool.tile([128, L], BF, tag=f"qt{oc}", name=f"qt{oc}")
                for oc in range(2)
            ]
            qslices = {}

            def emit_q_dma(blk):
                tiles = []
                for kc in range(KC):
                    xt = qxpool.tile([128, BLKW], BF, tag=f"qx{kc % 4}")
                    nc.sync.dma_start(
                        xt[:],
                        xq[kc * 128:(kc + 1) * 128, blk * BLKW:(blk + 1) * BLKW])
                    tiles.append(xt)
                qslices[blk] = tiles

            def emit_q_blk(blk):
                qps = spsum.tile([128, 2 * BLKW], F32, tag="s", name=f"qps{blk}")
                tiles = qslices.pop(blk)
                for kc in range(KC):
                    for oc in range(2):
                        nc.tensor.matmul(
                            qps[:, oc * BLKW:(oc + 1) * BLKW],
                            wq_sb[:, kc * GC + oc * 128: kc * GC + (oc + 1) * 128],
                            tiles[kc][:],
                            start=(kc == 0),
                            stop=(kc == KC - 1),
                        )
                dst0 = qt_sb[0][:, blk * BLKW:(blk + 1) * BLKW]
                dst1 = qt_sb[1][:, blk * BLKW:(blk + 1) * BLKW]
                nc.vector.tensor_scalar_add(dst0, qps[:, 0:BLKW], bq_sb[0][:])
                nc.scalar.activation(
                    dst1, qps[:, BLKW:2 * BLKW], Ident, bias=bq_sb[1][:]
                )

            emit_q_dma(0)
            emit_q_blk(0)

            if debug_dump:
                nc.sync.dma_start(dbg["kt0"][:], kt_sb[0][:])

            # V weights on the slow queue (needed by ~36us); W_o later
            nc.gpsimd.dma_start(wv_sb[:], wv[:])

            # q block-1 slices right behind block-0 so qt-blk1 is ready for
            # the B(2) pre-run that bridges the xv-DMA gap
            emit_q_dma(1)

            # ---- xv loads as 512-wide column slices into their own small
            # pool: V-proj unit lc only needs slice column-block lc//4 of
            # every channel chunk, so the V projection streams right behind
            # these DMAs instead of gating on full-chunk residency ----
            xvs = {}
            for vb in range(NBLK):
                for kc in range(KC):
                    t = xvspool.tile([128, BLKW], BF, tag=f"xvs{kc}", name="xvst")
                    nc.sync.dma_start(
                        t[:],
                        xv[kc * 128:(kc + 1) * 128, vb * BLKW:(vb + 1) * BLKW])
                    xvs[(kc, vb)] = t
            # q blocks 2-3 reuse the xvs tags (free once the V projection
            # consumes them); odd chunks ride the idle gpsimd queue
            for blk in (2, 3):
                tiles = []
                for kc in range(KC):
                    t = xvspool.tile([128, BLKW], BF, tag=f"xvs{kc}", name="qxl")
                    dma2(kc, t[:],
                         xq[kc * 128:(kc + 1) * 128, blk * BLKW:(blk + 1) * BLKW])
                    tiles.append(t)
                qslices[blk] = tiles

            nc.gpsimd.dma_start(wo_sb[:], wo[:])

            # dummy partition_broadcast: forces the GPSIMD custom-op
            # library load (~8us) to happen here, overlapped with the DMA
            # phase, instead of stalling the first softmax normalize
            dmb_i = wpool.tile([1, 8], F32, tag="dmb_i")
            nc.vector.memset(dmb_i[:], 0.0)
            dmb_o = wpool.tile([2, 8], F32, tag="dmb_o")
            nc.gpsimd.partition_broadcast(dmb_o[:], dmb_i[:])

            # ---- V projection (emitted inside main loop as filler) ----
            v_sb = []

            def emit_V():
                for lc in range(LC):
                    vps = ypsum.tile([128, BLKW], F32, tag="y", name="vps")
                    for kc in range(KC):
                        nc.tensor.matmul(
                            vps[:, 0:VW],
                            xvs[(kc, lc // 4)][:, (lc % 4) * 128:(lc % 4 + 1) * 128],
                            wv_sb[:, kc * VW:(kc + 1) * VW],
                            start=(kc == 0),
                            stop=(kc == KC - 1),
                        )
                    vt = vpool.tile([128, VW], BF, tag=f"v{lc}", name="vt")
                    nc.vector.tensor_add(vt[:], vps[:, 0:VW], vpat[:])
                    v_sb.append(vt)
                if debug_dump:
                    nc.sync.dma_start(dbg["v0"][:], v_sb[0][:])

            def emit_K_oc1():
                # K-oc1 column-blocked on the y-ring; chunks already
                # resident. Emitted AFTER B(0) so the scheduler slots these
                # matmuls into the exp-paced gaps of the first S phase
                # instead of blocking it.
                for lkb in range(NBLK):
                    kp = ypsum.tile([128, BLKW], F32, tag="y", name=f"kp1_{lkb}")
                    for kc in range(KC):
                        nc.tensor.matmul(
                            kp[:],
                            wk_sb[:, kc * GC + 128: kc * GC + 256],
                            xk_sb[(kc, lkb // 2)][:, (lkb % 2) * BLKW:(lkb % 2 + 1) * BLKW],
                            start=(kc == 0),
                            stop=(kc == KC - 1),
                        )
                    dst = kt_sb[1][:, lkb * BLKW:(lkb + 1) * BLKW]
                    if lkb % 2 == 0:
                        nc.vector.tensor_copy(dst, kp[:])
                    else:
                        nc.scalar.copy(dst, kp[:])

            # ---- main loop: attention + output projection ----
            ybar = [
                qkpool.tile([128, L], BF, tag=f"yb{oc}", name=f"yb{oc}")
                for oc in range(2)
            ]
            ptmaps = {}

            def emit_B(p, gstart=0, gend=16):
                blk, hp = divmod(p, 2)
                lq0 = blk * BLKW
                units = [(hh, lk) for lk in range(LC) for hh in range(2)]
                ptloc = ptmaps.setdefault(p, {})
                for g in range(gstart, gend):
                    grp = units[2 * g:2 * g + 2]
                    w = len(grp) * BLKW
                    dve = g in OFF_GROUPS
                    if dve:
                        ps = spsum.tile([128, 2 * BLKW], F32, tag="sd", bufs=1, name="pssd")
                    else:
                        ps = spsum.tile([128, 2 * BLKW], F32, tag="s", name="pss")
                    for i, (hh, lk) in enumerate(grp):
                        h = 2 * hp + hh
                        oc = h // 2
                        r0 = (h % 2) * 64
                        nc.tensor.matmul(
                            ps[:, i * BLKW:(i + 1) * BLKW],
                            kt_sb[oc][r0:r0 + 64, lk * 128:(lk + 1) * 128],
                            qt_sb[oc][r0:r0 + 64, lq0:lq0 + BLKW],
                            start=True,
                            stop=True,
                        )
                    pt = ptpool.tile([128, 2 * BLKW], BF, tag="pt")
                    if dve:
                        # DVE path: exp via Schraudolph bits, int16 written
                        # straight into the bf16 pt tile
                        nc.vector.tensor_scalar(
                            pt[:, 0:w].bitcast(I16),
                            ps[:, 0:w],
                            A_EXP,
                            B_EXP,
                            mybir.AluOpType.mult,
                            mybir.AluOpType.add,
                        )
                    else:
                        nc.scalar.activation(pt[:, 0:w], ps[:, 0:w], ExpF, scale=scale)
                    for i, (hh, lk) in enumerate(grp):
                        ptloc[(hh, lk)] = (pt, i * BLKW)

            def emit_C(p):
                blk, hp = divmod(p, 2)
                lq0 = blk * BLKW
                ptloc = ptmaps.pop(p)
                yexts = [
                    ypsum.tile([128, BLKW], F32, tag="y", name="yext")
                    for _ in range(2)
                ]
                for lk in range(LC):
                    for hh in range(2):
                        h = 2 * hp + hh
                        pt, col = ptloc[(hh, lk)]
                        nc.tensor.matmul(
                            yexts[hh][0:D + 1, :],
                            v_sb[lk][:, h * (D + 1):(h + 1) * (D + 1)],
                            pt[:, col:col + BLKW],
                            start=(lk == 0),
                            stop=(lk == LC - 1),
                        )
                for hh in range(2):
                    h = 2 * hp + hh
                    yext = yexts[hh]
                    if debug_dump and p == 0 and hh == 0:
                        ydump = smpool.tile([D + 1, BLKW], F32, tag="ydump")
                        nc.vector.tensor_copy(ydump[:], yext[0:D + 1, :])
                        nc.sync.dma_start(dbg["yext"][:], ydump[:])
                    # rs copy to SBUF: reciprocal_approx_fast is a raw-bit
                    # DVE trick and must not read PSUM directly (HW-verified
                    # failure: denominators come out garbage)
                    rs = smpool.tile([1, BLKW], F32, tag="rs")
                    nc.vector.tensor_copy(rs[:], yext[D:D + 1, :])
                    rr = smpool.tile([1, BLKW], F32, tag="rr")
                    nc.vector.reciprocal_approx_fast(rr[:], rs[:])
                    rbc = smpool.tile([64, BLKW], F32, tag="rbc")
                    nc.gpsimd.partition_broadcast(rbc[:], rr[:])
                    if debug_dump and p == 0 and hh == 0:
                        nc.sync.dma_start(dbg["rbc"][:], rbc[:])
                    oc = h // 2
                    r0 = (h % 2) * 64
                    nc.vector.tensor_mul(
                        ybar[oc][r0:r0 + 64, lq0:lq0 + BLKW],
                        yext[0:D, :],
                        rbc[:],
                    )

            def emit_D(blk):
                for lc in range(4 * blk, 4 * blk + 4):
                    osb = opool.tile([128, C], BF, tag="osb")
                    for cc in range(2):
                        po = ypsum.tile([128, BLKW], F32, tag="y")
                        for kc2 in range(2):
                            nc.tensor.matmul(
                                po[:],
                                ybar[kc2][:, lc * 128:(lc + 1) * 128],
                                wo_sb[:, kc2 * C + cc * BLKW: kc2 * C + (cc + 1) * BLKW],
                                start=(kc2 == 0),
                                stop=(kc2 == 1),
                            )
                        nc.vector.tensor_copy(
                            osb[:, cc * BLKW:(cc + 1) * BLKW], po[:]
                        )
                    nc.sync.dma_start(out[lc * 128:(lc + 1) * 128, :], osb[:])

            def emit_D_tail(blk):
                # tail variant: the S-phase is over, so its 6 psum banks
                # are free — run the last output-projection blocks there
                # (keeping the y-ring for PV(7)+normalize) with drains
                # split across DVE and the now-idle ScalarE
                for lc in range(4 * blk, 4 * blk + 4):
                    osb = opool.tile([128, C], BF, tag="osb")
                    dps = spsum.tile([128, 2 * BLKW], F32, tag="s", name=f"dp{lc}")
                    for cc in range(2):
                        for kc2 in range(2):
                            nc.tensor.matmul(
                                dps[:, cc * BLKW:(cc + 1) * BLKW],
                                ybar[kc2][:, lc * 128:(lc + 1) * 128],
                                wo_sb[:, kc2 * C + cc * BLKW: kc2 * C + (cc + 1) * BLKW],
                                start=(kc2 == 0),
                                stop=(kc2 == 1),
                            )
                    nc.vector.tensor_copy(osb[:, 0:BLKW], dps[:, 0:BLKW])
                    nc.scalar.copy(osb[:, BLKW:2 * BLKW], dps[:, BLKW:2 * BLKW])
                    nc.sync.dma_start(out[lc * 128:(lc + 1) * 128, :], osb[:])

            emit_B(0)
            emit_K_oc1()
            emit_B(1)
            # qt-blk1 + a 12-group pre-run of B(2) (8 groups): keeps both exp engines
            # fed across the window where the xv DMAs and V projection
            # gate PV(0) (the pt pool holds 16+16+8 tiles)
            emit_q_blk(1)
            emit_B(2, 0, 8)
            emit_V()   # V projection: PE filler under the first S/exp phases
            emit_C(0)
            emit_B(2, 8, 16)
            emit_C(1)
            for p in range(3, 8):
                if p % 2 == 0:
                    # qt block for this pair of S phases (data arrives via
                    # the column-sliced xq DMAs queued behind xv)
                    emit_q_blk(p // 2)
                emit_B(p)
                emit_C(p - 1)
                if p in (3, 5):
                    # D has a full block of slack: deprioritize it so it only
                    # fills PE gaps instead of preempting S^T production.
                    with tc.high_priority(offset=-400):
                        emit_D((p - 3) // 2)
            emit_C(7)
            emit_D_tail(2)
            emit_D_tail(3)

            if debug_dump:
                nc.sync.dma_start(dbg["qt0"][:], qt_sb[0][:])

    nc.compile()
    return nc


def _get_nc():
    if "nc" not in _CACHE:
        _CACHE["nc"] = _build()
    return _CACHE["nc"]


def _prep_in_maps(q, k, v, Wq, bq, Wk, bv_unused, Wv, Wo):
    del bv_unused
    xT = {}
    for b in range(B):
        xT[("q", b)] = np.ascontiguousarray(q[b].T).astype(BF16)
        xT[("k", b)] = np.ascontiguousarray(k[b].T).astype(BF16)
        xT[("v", b)] = np.ascontiguousarray(v[b].T).astype(BF16)
    in_maps = []
    for c in range(8):
        b, g = c // NG, c % NG
        wv_g = np.zeros((C, VW), dtype=BF16)
        for h in range(HPG):
            wv_g[:, h * (D + 1):h * (D + 1) + D] = Wv[
                :, g * GC + h * D: g * GC + (h + 1) * D
            ].astype(BF16)
        def tile_w(w):
            # [C_or_GC rows, n cols] -> [128, KC_chunks * n]: chunk kc holds
            # rows kc*128..kc*128+128 side by side
            r, n = w.shape
            return np.ascontiguousarray(
                w.reshape(r // 128, 128, n).transpose(1, 0, 2).reshape(128, -1)
            )
        in_maps.append({
            "xqT": xT[("q", b)],
            "xkT": xT[("k", b)],
            "xvT": xT[("v", b)],
            "wq": tile_w(Wq[:, g * GC:(g + 1) * GC].astype(BF16)),
            "wk": tile_w(Wk[:, g * GC:(g + 1) * GC].astype(BF16)),
            "wv": tile_w(wv_g),
            "wo": tile_w(Wo[g * GC:(g + 1) * GC, :].astype(BF16)),
            "bq": bq[g * GC:(g + 1) * GC].reshape(GC, 1).astype(np.float32),
        })
    return in_maps


def kernel(q, k, v, Wq, bq, Wk, bk, Wv, bv, Wo, bo):
    q = np.asarray(q, dtype=np.float32)
    k = np.asarray(k, dtype=np.float32)
    v = np.asarray(v, dtype=np.float32)
    Wq = np.asarray(Wq, dtype=np.float32)
    bq = np.asarray(bq, dtype=np.float32)
    Wk = np.asarray(Wk, dtype=np.float32)
    Wv = np.asarray(Wv, dtype=np.float32)
    bv = np.asarray(bv, dtype=np.float32)
    Wo = np.asarray(Wo, dtype=np.float32)
    bo = np.asarray(bo, dtype=np.float32)

    nc = _get_nc()
    in_maps = _prep_in_maps(q, k, v, Wq, bq, Wk, None, Wv, Wo)

    trace = bool(int(os.environ.get("BASS_KERNEL_PROFILE", "0")))
    kwargs = {}
    if trace:
        try:
            from ntff_hook import install as _install_hook
            _install_hook()
        except Exception:
            pass
        kwargs = {"trace": True}
        td = os.environ.get("BASS_KERNEL_TRACE_DIR")
        if td:
            kwargs["tmpdir"] = td
    res = run_bass_kernel_spmd(nc, in_maps, core_ids=list(range(8)), **kwargs)
    _CACHE["last_exec_time_ns"] = res.exec_time_ns

    # host gather: sum group partials per batch, add folded bias
    cvec = (bv.astype(np.float64) @ Wo.astype(np.float64) + bo).astype(np.float32)
    full = np.empty((B, L, C), dtype=np.float32)
    for b in range(B):
        acc = res.results[b * NG]["out"].astype(np.float32)
        for g in range(1, NG):
            acc = acc + res.results[b * NG + g]["out"].astype(np.float32)
        full[b] = acc + cvec[None, :]
    return full
